# revision 1
# baseline (speedup 1.0000x reference)
"""CachedAttention decode kernel for 8 TRN2 NeuronCores.

Problem: single-position cached attention (decode step).
  x:[16,1,2048], cache_k/v:[16,16,4096,128], W_q/k/v/o:[2048,2048] (torch
  Linear convention: y = x @ W.T).

Sharding: head-parallel across 8 cores, 2 heads/core. W_q/W_k/W_v
column-parallel (each core projects only its heads), W_o row-parallel
(each core computes a partial [16,2048] output; host sums the 8 partials).

Per-core device algorithm (all 16 batches, 2 heads):
  - projections computed TRANSPOSED: qT = Wq_rows @ x^T -> [d, b] per head,
    so q lands with head_dim on partitions (no on-chip transposes anywhere).
  - K cache is staged host-side as K^T [d, s] per (h,b); QK matmul uses
    K^T-tile as the stationary operand, q column as moving -> scores land
    [s_tile, 1] in PSUM with s on partitions.
  - softmax without max-subtraction (scores ~ N(0,1), exp is safe), exp on
    the scalar engine with the 1/sqrt(D) scale folded in.
  - PV: V natural [s, d] tiles stationary, p column moving, accumulated in
    PSUM -> context [d, b] per head.
  - the appended new position (k,v of the current token) is folded in as a
    rank-1 update batched over all (h,b) via a ones-broadcast matmul.
  - W_o partial: lhsT = normalized context [d, b], rhs = W_o^T slice.

KV cache is cast to bf16 on host (halves HBM traffic; scores/psum stay
fp32). Set DT_KV = mybir.dt.float32 below for a full-fp32 variant.
"""
import sys

sys.path.insert(0, "/opt/trn_rl_repo")

from contextlib import ExitStack

import numpy as np

import concourse.bass as bass
import concourse.tile as tile
from concourse import bacc, mybir
from concourse.bass_utils import run_bass_kernel_spmd

# ---- problem constants (hardcoded; kernel.py must be self-contained) ----
B = 16          # batch
H = 16          # total heads
S = 4096        # cached sequence length
D = 128         # head dim
DM = 2048       # d_model
N_CORES = 8
HPC = H // N_CORES   # heads per core = 2
G = HPC * B          # (head, batch) pairs per core = 32
ST = S // 128        # s-tiles per (h,b) = 32
CH = 4               # max batches per KV DMA chunk (4 MiB transfers)
NG = B // CH         # chunk groups per head (host-packed contiguous 4 MiB)
KT = 16              # k-tiles over d_model contraction
SCALE = float(D) ** -0.5

F32 = mybir.dt.float32
DT_KV = mybir.dt.bfloat16


def _build_kernel():
    nc = bacc.Bacc("TRN2", target_bir_lowering=False, debug=False)

    # DRAM parameters (per-core shards, host-prepared layouts)
    kt_d = nc.declare_dram_parameter("kt", [HPC, NG, 128, CH * S], DT_KV, isOutput=False)
    vv_d = nc.declare_dram_parameter("vv", [HPC, NG, 128, CH * S], DT_KV, isOutput=False)
    wq_d = nc.declare_dram_parameter("wq", [128, KT * HPC * D], DT_KV, isOutput=False)
    wk_d = nc.declare_dram_parameter("wk", [128, KT * HPC * D], DT_KV, isOutput=False)
    wv_d = nc.declare_dram_parameter("wv", [128, KT * HPC * D], DT_KV, isOutput=False)
    wo_d = nc.declare_dram_parameter("wo", [128, HPC * DM], DT_KV, isOutput=False)
    xt_d = nc.declare_dram_parameter("xt", [128, KT * B], DT_KV, isOutput=False)
    out_d = nc.declare_dram_parameter("out", [B, DM], F32, isOutput=True)

    with tile.TileContext(nc) as tc, ExitStack() as ctx:
        wpool = ctx.enter_context(tc.tile_pool(name="w", bufs=1))
        spool = ctx.enter_context(tc.tile_pool(name="s", bufs=1))
        kpool = ctx.enter_context(tc.tile_pool(name="k", bufs=2))
        vpool = ctx.enter_context(tc.tile_pool(name="v", bufs=2))
        ppool = ctx.enter_context(tc.tile_pool(name="p", bufs=2 * CH))
        epool = ctx.enter_context(tc.tile_pool(name="e", bufs=2))
        ps_sc = ctx.enter_context(tc.tile_pool(name="psc", bufs=2, space="PSUM"))
        ps_cx = ctx.enter_context(tc.tile_pool(name="pcx", bufs=2, space="PSUM"))
        ps_ms = ctx.enter_context(tc.tile_pool(name="pms", bufs=2, space="PSUM"))
        ps_wo = ctx.enter_context(tc.tile_pool(name="pwo", bufs=2, space="PSUM"))

        # resident weights / activations. xt + wq gate the first projection
        # matmuls, so they go FIRST on the sync ring (ahead of the K-chunk
        # stream); wo isn't needed until the h=0 epilogue and rides the
        # scalar ring ahead of the V chunks.
        # Weights ride the gpsimd (SWDGE) DMA path so both HWDGE rings carry
        # nothing but the KV stream from t=0.
        xt_sb = wpool.tile([128, KT * B], DT_KV, tag="xt")
        nc.gpsimd.dma_start(xt_sb[:], xt_d[:])
        wq_sb = wpool.tile([128, KT * HPC * D], DT_KV, tag="wq")
        nc.gpsimd.dma_start(wq_sb[:], wq_d[:])
        wk_sb = wpool.tile([128, KT * HPC * D], DT_KV, tag="wk")
        nc.gpsimd.dma_start(wk_sb[:], wk_d[:])
        wv_sb = wpool.tile([128, KT * HPC * D], DT_KV, tag="wv")
        nc.gpsimd.dma_start(wv_sb[:], wv_d[:])
        wo_sb = wpool.tile([128, HPC * DM], DT_KV, tag="wo")
        nc.gpsimd.dma_start(wo_sb[:], wo_d[:])

        ones_bf = spool.tile([128, 1], DT_KV, tag="ones_bf")
        nc.vector.memset(ones_bf[:], 1.0)
        ones1 = spool.tile([1, 128], F32, tag="ones1")
        nc.vector.memset(ones1[:], 1.0)

        q_bf = spool.tile([128, G], DT_KV, tag="q_bf")
        knew_bf = spool.tile([128, G], DT_KV, tag="knew_bf")
        vnewT = spool.tile([128, G], F32, tag="vnewT")
        p_new = spool.tile([1, G], F32, tag="p_new")
        denom = spool.tile([1, G], F32, tag="denom")
        dtot = spool.tile([1, G], F32, tag="dtot")
        recip = spool.tile([1, G], F32, tag="recip")
        out_sb = spool.tile([B, DM], F32, tag="out_sb")

        # ---- projections, transposed: proj[d, b] per head ----
        def emit_proj(w_sb, dst):
            for h in range(HPC):
                pr_ps = ps_ms.tile([128, B], F32, tag="misc", name=f"pr_{h}")
                for kk in range(KT):
                    nc.tensor.matmul(
                        pr_ps[:],
                        w_sb[:, kk * HPC * D + h * D: kk * HPC * D + (h + 1) * D],
                        xt_sb[:, kk * B: (kk + 1) * B],
                        start=(kk == 0), stop=(kk == KT - 1),
                    )
                nc.scalar.copy(dst[:, h * B: (h + 1) * B], pr_ps[:])

        # q-projection gates the very first QK matmuls — emit it first. The
        # k/v projections + new-position scores are only needed by the first
        # epilogue (~chunk 15), so they're emitted after chunk 1 to keep the
        # early PE program free of waits on wk/wv.
        emit_proj(wq_sb, q_bf)

        def emit_kv_proj_and_snew():
            emit_proj(wk_sb, knew_bf)
            emit_proj(wv_sb, vnewT)
            sn_ps = ps_ms.tile([1, G], F32, tag="misc")
            for g in range(G):
                nc.tensor.matmul(
                    sn_ps[:, g: g + 1],
                    knew_bf[:, g: g + 1],
                    q_bf[:, g: g + 1],
                    start=True, stop=True,
                )
            nc.scalar.activation(p_new[:], sn_ps[:],
                                 mybir.ActivationFunctionType.Exp, scale=SCALE)

        # ---- main attention loop, software-pipelined by one chunk ----
        # K chunks ride the sync DMA ring, V chunks the scalar ring. PE
        # emission order per steady-state iteration:
        #   QK(chunk n) ; PV(chunk n-1) ; denom(chunk n-1)
        # so the PE never stalls on the exp (ACT) latency of chunk n or on
        # chunk n's V arrival — it always has a chunk-old PV group ready.
        ctx_tiles = {}

        def emit_pv(ph, b0, cc, pv_sb, plist):
            ctx_ps = ctx_tiles[ph]
            for bl in range(cc):
                b = b0 + bl
                for si in range(ST):
                    nc.tensor.matmul(
                        ctx_ps[:, b: b + 1],
                        pv_sb[:, bl * S + si * 128: bl * S + (si + 1) * 128],
                        plist[bl][:, si: si + 1],
                        start=(si == 0), stop=(si == ST - 1),
                    )
            for bl in range(cc):
                g = ph * B + b0 + bl
                dn_ps = ps_ms.tile([1, ST], F32, tag="misc")
                nc.tensor.matmul(dn_ps[:], ones_bf[:], plist[bl][:],
                                 start=True, stop=True)
                nc.vector.reduce_sum(denom[:, g: g + 1], dn_ps[:],
                                     axis=mybir.AxisListType.X)

        def emit_epilogue_pre(h):
            # Everything except the W_o matmuls — runs on ACT/GpSimd/DVE so
            # the PE pipeline is never blocked on this serial chain.
            ctx_ps = ctx_tiles[h]
            hs = slice(h * B, (h + 1) * B)
            ctx_sb = epool.tile([128, B], F32, tag="ctx_sb")
            nc.scalar.copy(ctx_sb[:], ctx_ps[:])
            # + p_new * v_new  (rank-1 new-position update, batched over b)
            pb_bc = epool.tile([128, B], F32, tag="pb_bc")
            nc.gpsimd.partition_broadcast(pb_bc[:], p_new[:, hs])
            nt = epool.tile([128, B], F32, tag="nt")
            nc.vector.tensor_mul(nt[:], vnewT[:, hs], pb_bc[:])
            nc.vector.tensor_add(ctx_sb[:], ctx_sb[:], nt[:])
            # normalize by (denom + p_new)
            nc.vector.tensor_add(dtot[:, hs], denom[:, hs], p_new[:, hs])
            nc.vector.reciprocal(recip[:, hs], dtot[:, hs])
            rb_bc = epool.tile([128, B], F32, tag="rb_bc")
            nc.gpsimd.partition_broadcast(rb_bc[:], recip[:, hs])
            ctx_n = epool.tile([128, B], DT_KV, tag=f"ctx_n{h}", name=f"ctx_n{h}")
            nc.vector.tensor_mul(ctx_n[:], ctx_sb[:], rb_bc[:])
            return ctx_n

        def emit_epilogue_wo(h, ctx_n):
            # W_o partial: out[b, j] += sum_d ctx_n[d, b] * WoT[h*128+d, j]
            for nchk in range(DM // 512):
                wo_ps = ps_wo.tile([B, 512], F32, tag="wo")
                nc.tensor.matmul(
                    wo_ps[:],
                    ctx_n[:],
                    wo_sb[:, h * DM + nchk * 512: h * DM + (nchk + 1) * 512],
                    start=True, stop=True,
                )
                if h == 0:
                    nc.scalar.copy(out_sb[:, nchk * 512: (nchk + 1) * 512], wo_ps[:])
                else:
                    nc.vector.tensor_add(out_sb[:, nchk * 512: (nchk + 1) * 512],
                                         out_sb[:, nchk * 512: (nchk + 1) * 512],
                                         wo_ps[:])

        pend = None
        wo_pend = None
        idx = 0
        for h in range(HPC):
            ctx_tiles[h] = ps_cx.tile([128, B], F32, tag="ctx", name=f"ctx_{h}")
            b0 = 0
            for ng in range(NG):
                cc = CH
                kt_sb = kpool.tile([128, CH * S], DT_KV, tag="kt")
                nc.sync.dma_start(kt_sb[:], kt_d[h, ng])
                v_sb = vpool.tile([128, CH * S], DT_KV, tag="vt")
                nc.scalar.dma_start(v_sb[:], vv_d[h, ng])
                plist = []
                for bl in range(cc):
                    g = h * B + b0 + bl
                    sc_ps = ps_sc.tile([128, ST], F32, tag="sc")
                    for si in range(ST):
                        nc.tensor.matmul(
                            sc_ps[:, si: si + 1],
                            kt_sb[:, bl * S + si * 128: bl * S + (si + 1) * 128],
                            q_bf[:, g: g + 1],
                            start=True, stop=True,
                        )
                    p_sb = ppool.tile([128, ST], DT_KV, tag="p")
                    nc.scalar.activation(p_sb[:], sc_ps[:],
                                         mybir.ActivationFunctionType.Exp,
                                         scale=SCALE)
                    plist.append(p_sb)
                if pend is not None:
                    emit_pv(*pend)
                    if wo_pend is not None:
                        emit_epilogue_wo(*wo_pend)
                        wo_pend = None
                    if pend[0] != h:
                        wo_pend = (pend[0], emit_epilogue_pre(pend[0]))
                pend = (h, b0, cc, v_sb, plist)
                if idx == 1:
                    emit_kv_proj_and_snew()
                idx += 1
                b0 += cc
        emit_pv(*pend)
        wo_pend2 = (HPC - 1, emit_epilogue_pre(HPC - 1))
        if wo_pend is not None:
            emit_epilogue_wo(*wo_pend)
        emit_epilogue_wo(*wo_pend2)

        nc.sync.dma_start(out_d[:], out_sb[:])

    nc.finalize()
    return nc


_NC_CACHE = None


def _get_kernel():
    global _NC_CACHE
    if _NC_CACHE is None:
        _NC_CACHE = _build_kernel()
    return _NC_CACHE


def _np_kv(a):
    return np.ascontiguousarray(a, dtype=mybir.dt.np(DT_KV))


def _shard_inputs(x, cache_k, cache_v, W_q, W_k, W_v, W_o):
    """Build per-core input maps with the on-device layouts."""
    x = np.asarray(x, dtype=np.float32)
    cache_k = np.asarray(cache_k, dtype=np.float32)
    cache_v = np.asarray(cache_v, dtype=np.float32)
    W_q = np.asarray(W_q, dtype=np.float32)
    W_k = np.asarray(W_k, dtype=np.float32)
    W_v = np.asarray(W_v, dtype=np.float32)
    W_o = np.asarray(W_o, dtype=np.float32)

    # xt[p, kk*B + b] = x[b, 0, kk*128 + p]  (shared by all cores)
    xt = _np_kv(
        x[:, 0, :].T.reshape(KT, 128, B).transpose(1, 0, 2).reshape(128, KT * B)
    )

    in_maps = []
    for c in range(N_CORES):
        rows = slice(c * HPC * D, (c + 1) * HPC * D)
        # K^T per (h,b): [d, s]; pack CH batches along free dim per chunk
        k_c = cache_k[:, c * HPC:(c + 1) * HPC]          # [B, HPC, S, D]
        k_t = k_c.transpose(1, 0, 3, 2)                  # [HPC, B, D, S]
        k_t = k_t.reshape(HPC, NG, CH, 128, S).transpose(0, 1, 3, 2, 4)
        k_t = k_t.reshape(HPC, NG, 128, CH * S)
        # V natural per (h,b): rows s in tiles of 128 on partitions:
        # v[h, b, p, si*128 + d] = V[si*128 + p, d]
        v_c = cache_v[:, c * HPC:(c + 1) * HPC]          # [B, HPC, S, D]
        v_t = v_c.transpose(1, 0, 2, 3)                  # [HPC, B, S, D]
        v_t = v_t.reshape(HPC, B, ST, 128, D).transpose(0, 1, 3, 2, 4)
        v_t = v_t.reshape(HPC, NG, CH, 128, ST * D).transpose(0, 1, 3, 2, 4)
        v_t = v_t.reshape(HPC, NG, 128, CH * S)

        def wslice(W):
            # w[p, kk*HPC*D + h*D + m] = W[rows][h*D + m, kk*128 + p]
            wr = W[rows, :]                              # [HPC*D, DM]
            wr = wr.reshape(HPC * D, KT, 128).transpose(2, 1, 0)   # [p, kk, m]
            return _np_kv(wr.reshape(128, KT * HPC * D))

        # wo[p, h*DM + j] = W_o[j, c*HPC*D + h*128 + p]
        wo = W_o[:, rows].T.reshape(HPC, 128, DM).transpose(1, 0, 2)
        wo = _np_kv(wo.reshape(128, HPC * DM))

        in_maps.append({
            "kt": _np_kv(k_t),
            "vv": _np_kv(v_t),
            "wq": wslice(W_q),
            "wk": wslice(W_k),
            "wv": wslice(W_v),
            "wo": wo,
            "xt": xt,
        })
    return in_maps


def run_sharded(inputs, trace=False):
    """Run the SPMD kernel; returns (list of per-core partials, BassKernelResults)."""
    nc = _get_kernel()
    in_maps = _shard_inputs(**inputs)
    res = run_bass_kernel_spmd(nc, in_maps, core_ids=list(range(N_CORES)),
                               trace=trace)
    return res


def kernel(x, cache_k, cache_v, W_q, W_k, W_v, W_o) -> np.ndarray:
    res = run_sharded(dict(x=x, cache_k=cache_k, cache_v=cache_v,
                           W_q=W_q, W_k=W_k, W_v=W_v, W_o=W_o))
    total = np.zeros((B, DM), dtype=np.float32)
    for c in range(N_CORES):
        total += res.results[c]["out"]
    return total.reshape(B, 1, DM)



# revision 2
# speedup vs baseline: 1.4969x; 1.4969x over previous
"""CachedAttention decode kernel for 8 TRN2 NeuronCores.

Problem: single-position cached attention (decode step).
  x:[16,1,2048], cache_k/v:[16,16,4096,128], W_q/k/v/o:[2048,2048] (torch
  Linear convention: y = x @ W.T).

Sharding: head-parallel across 8 cores, 2 heads/core. W_q/W_k/W_v
column-parallel (each core projects only its heads), W_o row-parallel
(each core computes a partial [16,2048] output; host sums the 8 partials).

Per-core device algorithm (all 16 batches, 2 heads):
  - projections computed TRANSPOSED: qT = Wq_rows @ x^T -> [d, b] per head,
    so q lands with head_dim on partitions (no on-chip transposes anywhere).
  - K cache is staged host-side as K^T [d, s] per (h,b); QK matmul uses
    K^T-tile as the stationary operand, q column as moving -> scores land
    [s_tile, 1] in PSUM with s on partitions.
  - softmax without max-subtraction (scores ~ N(0,1), exp is safe), exp on
    the scalar engine with the 1/sqrt(D) scale folded in.
  - PV: V natural [s, d] tiles stationary, p column moving, accumulated in
    PSUM -> context [d, b] per head.
  - the appended new position (k,v of the current token) is folded in as a
    rank-1 update batched over all (h,b) via a ones-broadcast matmul.
  - W_o partial: lhsT = normalized context [d, b], rhs = W_o^T slice.

Precision plan: KV cache stored as fp8 e3m4 (float8e3) scaled by 2.0 so
N(0,1) data fills the format's normal range (max 15.5) -- halves HBM
traffic vs bf16 again. The x2 score scale is folded into the exp scale
(SCALE/2); the x2 on V is cancelled by scaling W_o by 0.5 host-side (and
W_v by 2 so the new-position term matches). Everything else (x, weights,
q, p) rides fp16 (better mantissa than bf16, same speed); scores/psum
stay fp32.
"""
import sys

sys.path.insert(0, "/opt/trn_rl_repo")

from contextlib import ExitStack

import numpy as np

import concourse.bass as bass
import concourse.tile as tile
from concourse import bacc, mybir
from concourse.bass_utils import run_bass_kernel_spmd

# ---- problem constants (hardcoded; kernel.py must be self-contained) ----
B = 16          # batch
H = 16          # total heads
S = 4096        # cached sequence length
D = 128         # head dim
DM = 2048       # d_model
N_CORES = 8
HPC = H // N_CORES   # heads per core = 2
G = HPC * B          # (head, batch) pairs per core = 32
ST = S // 128        # s-tiles per (h,b) = 32
CH = 4               # max batches per KV DMA chunk
NG = B // CH         # chunk groups per head (host-packed contiguous)
KT = 16              # k-tiles over d_model contraction
SCALE = float(D) ** -0.5

F32 = mybir.dt.float32
DT_C = mybir.dt.float8e3     # KV cache storage (e3m4, scaled by KV_SCALE)
DT_A = mybir.dt.float16      # activations / weights / p
KV_SCALE = 2.0


def _build_kernel():
    nc = bacc.Bacc("TRN2", target_bir_lowering=False, debug=False)

    # DRAM parameters (per-core shards, host-prepared layouts)
    kt_d = nc.declare_dram_parameter("kt", [HPC, NG, 128, CH * S], DT_C, isOutput=False)
    vv_d = nc.declare_dram_parameter("vv", [HPC, NG, 128, CH * S], DT_C, isOutput=False)
    wq_d = nc.declare_dram_parameter("wq", [128, KT * HPC * D], DT_A, isOutput=False)
    wk_d = nc.declare_dram_parameter("wk", [128, KT * HPC * D], DT_A, isOutput=False)
    wv_d = nc.declare_dram_parameter("wv", [128, KT * HPC * D], DT_A, isOutput=False)
    wo_d = nc.declare_dram_parameter("wo", [128, HPC * DM], DT_A, isOutput=False)
    xt_d = nc.declare_dram_parameter("xt", [128, KT * B], DT_A, isOutput=False)
    out_d = nc.declare_dram_parameter("out", [B, DM], F32, isOutput=True)

    with tile.TileContext(nc) as tc, ExitStack() as ctx:
        wpool = ctx.enter_context(tc.tile_pool(name="w", bufs=1))
        spool = ctx.enter_context(tc.tile_pool(name="s", bufs=1))
        kpool = ctx.enter_context(tc.tile_pool(name="k", bufs=3))
        vpool = ctx.enter_context(tc.tile_pool(name="v", bufs=3))
        ppool = ctx.enter_context(tc.tile_pool(name="p", bufs=2 * CH))
        epool = ctx.enter_context(tc.tile_pool(name="e", bufs=2))
        ps_sc = ctx.enter_context(tc.tile_pool(name="psc", bufs=2, space="PSUM"))
        ps_cx = ctx.enter_context(tc.tile_pool(name="pcx", bufs=2, space="PSUM"))
        ps_ms = ctx.enter_context(tc.tile_pool(name="pms", bufs=2, space="PSUM"))
        ps_wo = ctx.enter_context(tc.tile_pool(name="pwo", bufs=2, space="PSUM"))

        # resident weights / activations. xt + wq gate the first projection
        # matmuls; weights ride the gpsimd (SWDGE) DMA path so both HWDGE
        # rings carry nothing but the KV stream from t=0.
        xt_sb = wpool.tile([128, KT * B], DT_A, tag="xt")
        nc.gpsimd.dma_start(xt_sb[:], xt_d[:])
        wq_sb = wpool.tile([128, KT * HPC * D], DT_A, tag="wq")
        nc.gpsimd.dma_start(wq_sb[:], wq_d[:])
        wk_sb = wpool.tile([128, KT * HPC * D], DT_A, tag="wk")
        nc.gpsimd.dma_start(wk_sb[:], wk_d[:])
        wv_sb = wpool.tile([128, KT * HPC * D], DT_A, tag="wv")
        nc.gpsimd.dma_start(wv_sb[:], wv_d[:])
        wo_sb = wpool.tile([128, HPC * DM], DT_A, tag="wo")
        nc.gpsimd.dma_start(wo_sb[:], wo_d[:])

        ones_a = spool.tile([128, 1], DT_A, tag="ones_a")
        nc.vector.memset(ones_a[:], 1.0)

        q_a = spool.tile([128, G], DT_A, tag="q_a")
        knew_a = spool.tile([128, G], DT_A, tag="knew_a")
        vnewT = spool.tile([128, G], F32, tag="vnewT")
        p_new = spool.tile([1, G], F32, tag="p_new")
        denom = spool.tile([1, G], F32, tag="denom")
        dtot = spool.tile([1, G], F32, tag="dtot")
        recip = spool.tile([1, G], F32, tag="recip")
        out_sb = spool.tile([B, DM], F32, tag="out_sb")

        # ---- projections, transposed: proj[d, b] per head ----
        def emit_proj(w_sb, dst):
            for h in range(HPC):
                pr_ps = ps_ms.tile([128, B], F32, tag="misc", name=f"pr_{h}")
                for kk in range(KT):
                    nc.tensor.matmul(
                        pr_ps[:],
                        w_sb[:, kk * HPC * D + h * D: kk * HPC * D + (h + 1) * D],
                        xt_sb[:, kk * B: (kk + 1) * B],
                        start=(kk == 0), stop=(kk == KT - 1),
                    )
                nc.scalar.copy(dst[:, h * B: (h + 1) * B], pr_ps[:])

        # q-projection gates the very first QK matmuls -- emit it first.
        emit_proj(wq_sb, q_a)

        def emit_kv_proj_and_snew():
            emit_proj(wk_sb, knew_a)
            emit_proj(wv_sb, vnewT)
            sn_ps = ps_ms.tile([1, G], F32, tag="misc")
            for g in range(G):
                nc.tensor.matmul(
                    sn_ps[:, g: g + 1],
                    knew_a[:, g: g + 1],
                    q_a[:, g: g + 1],
                    start=True, stop=True,
                )
            nc.scalar.activation(p_new[:], sn_ps[:],
                                 mybir.ActivationFunctionType.Exp, scale=SCALE)

        # ---- main attention loop, software-pipelined by one chunk ----
        ctx_tiles = {}

        def emit_pv(ph, b0, cc, pv_sb, plist):
            ctx_ps = ctx_tiles[ph]
            for bl in range(cc):
                b = b0 + bl
                for si in range(ST):
                    nc.tensor.matmul(
                        ctx_ps[:, b: b + 1],
                        pv_sb[:, bl * S + si * 128: bl * S + (si + 1) * 128],
                        plist[bl][:, si: si + 1],
                        start=(si == 0), stop=(si == ST - 1),
                    )
            for bl in range(cc):
                g = ph * B + b0 + bl
                dn_ps = ps_ms.tile([1, ST], F32, tag="misc")
                nc.tensor.matmul(dn_ps[:], ones_a[:], plist[bl][:],
                                 start=True, stop=True)
                nc.vector.reduce_sum(denom[:, g: g + 1], dn_ps[:],
                                     axis=mybir.AxisListType.X)

        def emit_epilogue_pre(h):
            # Everything except the W_o matmuls -- ACT/GpSimd/DVE only.
            ctx_ps = ctx_tiles[h]
            hs = slice(h * B, (h + 1) * B)
            ctx_sb = epool.tile([128, B], F32, tag="ctx_sb")
            nc.scalar.copy(ctx_sb[:], ctx_ps[:])
            # + p_new * v_new  (vnewT carries the x2 KV scale via W_v)
            pb_bc = epool.tile([128, B], F32, tag="pb_bc")
            nc.gpsimd.partition_broadcast(pb_bc[:], p_new[:, hs])
            nt = epool.tile([128, B], F32, tag="nt")
            nc.vector.tensor_mul(nt[:], vnewT[:, hs], pb_bc[:])
            nc.vector.tensor_add(ctx_sb[:], ctx_sb[:], nt[:])
            # normalize by (denom + p_new); the x2 on ctx cancels via W_o*0.5
            nc.vector.tensor_add(dtot[:, hs], denom[:, hs], p_new[:, hs])
            nc.vector.reciprocal(recip[:, hs], dtot[:, hs])
            rb_bc = epool.tile([128, B], F32, tag="rb_bc")
            nc.gpsimd.partition_broadcast(rb_bc[:], recip[:, hs])
            ctx_n = epool.tile([128, B], DT_A, tag=f"ctx_n{h}", name=f"ctx_n{h}")
            nc.vector.tensor_mul(ctx_n[:], ctx_sb[:], rb_bc[:])
            return ctx_n

        def emit_epilogue_wo(h, ctx_n):
            for nchk in range(DM // 512):
                wo_ps = ps_wo.tile([B, 512], F32, tag="wo")
                nc.tensor.matmul(
                    wo_ps[:],
                    ctx_n[:],
                    wo_sb[:, h * DM + nchk * 512: h * DM + (nchk + 1) * 512],
                    start=True, stop=True,
                )
                if h == 0:
                    nc.scalar.copy(out_sb[:, nchk * 512: (nchk + 1) * 512], wo_ps[:])
                else:
                    nc.vector.tensor_add(out_sb[:, nchk * 512: (nchk + 1) * 512],
                                         out_sb[:, nchk * 512: (nchk + 1) * 512],
                                         wo_ps[:])

        pend = None
        wo_pend = None
        idx = 0
        for h in range(HPC):
            ctx_tiles[h] = ps_cx.tile([128, B], F32, tag="ctx", name=f"ctx_{h}")
            b0 = 0
            for ng in range(NG):
                cc = CH
                kt_sb = kpool.tile([128, CH * S], DT_C, tag="kt")
                nc.sync.dma_start(kt_sb[:], kt_d[h, ng])
                v_sb = vpool.tile([128, CH * S], DT_C, tag="vt")
                nc.scalar.dma_start(v_sb[:], vv_d[h, ng])
                plist = []
                for bl in range(cc):
                    g = h * B + b0 + bl
                    sc_ps = ps_sc.tile([128, ST], F32, tag="sc")
                    for si in range(ST):
                        nc.tensor.matmul(
                            sc_ps[:, si: si + 1],
                            kt_sb[:, bl * S + si * 128: bl * S + (si + 1) * 128],
                            q_a[:, g: g + 1],
                            start=True, stop=True,
                        )
                    p_sb = ppool.tile([128, ST], DT_A, tag="p")
                    # scores carry the x{KV_SCALE} from the stored K
                    nc.scalar.activation(p_sb[:], sc_ps[:],
                                         mybir.ActivationFunctionType.Exp,
                                         scale=SCALE / KV_SCALE)
                    plist.append(p_sb)
                if pend is not None:
                    emit_pv(*pend)
                    if wo_pend is not None:
                        emit_epilogue_wo(*wo_pend)
                        wo_pend = None
                    if pend[0] != h:
                        wo_pend = (pend[0], emit_epilogue_pre(pend[0]))
                pend = (h, b0, cc, v_sb, plist)
                if idx == 1:
                    emit_kv_proj_and_snew()
                idx += 1
                b0 += cc
        emit_pv(*pend)
        wo_pend2 = (HPC - 1, emit_epilogue_pre(HPC - 1))
        if wo_pend is not None:
            emit_epilogue_wo(*wo_pend)
        emit_epilogue_wo(*wo_pend2)

        nc.sync.dma_start(out_d[:], out_sb[:])

    nc.finalize()
    return nc


_NC_CACHE = None


def _get_kernel():
    global _NC_CACHE
    if _NC_CACHE is None:
        _NC_CACHE = _build_kernel()
    return _NC_CACHE


def _np_c(a):
    # KV cache quantization: scale into e3m4's normal range, clip for safety
    return np.clip(a * KV_SCALE, -15.0, 15.0).astype(mybir.dt.np(DT_C))


def _np_a(a):
    return np.ascontiguousarray(a, dtype=mybir.dt.np(DT_A))


def _shard_inputs(x, cache_k, cache_v, W_q, W_k, W_v, W_o):
    """Build per-core input maps with the on-device layouts."""
    x = np.asarray(x, dtype=np.float32)
    cache_k = np.asarray(cache_k, dtype=np.float32)
    cache_v = np.asarray(cache_v, dtype=np.float32)
    W_q = np.asarray(W_q, dtype=np.float32)
    W_k = np.asarray(W_k, dtype=np.float32)
    # fold the KV_SCALE bookkeeping into the projection weights:
    #   vnew must carry the same x2 as the stored V cache -> W_v * 2
    #   the x2 on the whole context is cancelled at the end -> W_o * 0.5
    W_v = np.asarray(W_v, dtype=np.float32) * KV_SCALE
    W_o = np.asarray(W_o, dtype=np.float32) * (1.0 / KV_SCALE)

    # xt[p, kk*B + b] = x[b, 0, kk*128 + p]  (shared by all cores)
    xt = _np_a(
        x[:, 0, :].T.reshape(KT, 128, B).transpose(1, 0, 2).reshape(128, KT * B)
    )

    in_maps = []
    for c in range(N_CORES):
        rows = slice(c * HPC * D, (c + 1) * HPC * D)
        # K^T per (h,b): [d, s]; pack CH batches along free dim per chunk
        k_c = cache_k[:, c * HPC:(c + 1) * HPC]          # [B, HPC, S, D]
        k_t = k_c.transpose(1, 0, 3, 2)                  # [HPC, B, D, S]
        k_t = k_t.reshape(HPC, NG, CH, 128, S).transpose(0, 1, 3, 2, 4)
        k_t = k_t.reshape(HPC, NG, 128, CH * S)
        # V natural per (h,b): rows s in tiles of 128 on partitions:
        # v[h, b, p, si*128 + d] = V[si*128 + p, d]
        v_c = cache_v[:, c * HPC:(c + 1) * HPC]          # [B, HPC, S, D]
        v_t = v_c.transpose(1, 0, 2, 3)                  # [HPC, B, S, D]
        v_t = v_t.reshape(HPC, B, ST, 128, D).transpose(0, 1, 3, 2, 4)
        v_t = v_t.reshape(HPC, NG, CH, 128, ST * D).transpose(0, 1, 3, 2, 4)
        v_t = v_t.reshape(HPC, NG, 128, CH * S)

        def wslice(W):
            # w[p, kk*HPC*D + h*D + m] = W[rows][h*D + m, kk*128 + p]
            wr = W[rows, :]                              # [HPC*D, DM]
            wr = wr.reshape(HPC * D, KT, 128).transpose(2, 1, 0)   # [p, kk, m]
            return _np_a(wr.reshape(128, KT * HPC * D))

        # wo[p, h*DM + j] = W_o[j, c*HPC*D + h*128 + p]
        wo = W_o[:, rows].T.reshape(HPC, 128, DM).transpose(1, 0, 2)
        wo = _np_a(wo.reshape(128, HPC * DM))

        in_maps.append({
            "kt": _np_c(k_t),
            "vv": _np_c(v_t),
            "wq": wslice(W_q),
            "wk": wslice(W_k),
            "wv": wslice(W_v),
            "wo": wo,
            "xt": xt,
        })
    return in_maps


def run_sharded(inputs, trace=False):
    """Run the SPMD kernel; returns BassKernelResults."""
    nc = _get_kernel()
    in_maps = _shard_inputs(**inputs)
    res = run_bass_kernel_spmd(nc, in_maps, core_ids=list(range(N_CORES)),
                               trace=trace)
    return res


def kernel(x, cache_k, cache_v, W_q, W_k, W_v, W_o) -> np.ndarray:
    res = run_sharded(dict(x=x, cache_k=cache_k, cache_v=cache_v,
                           W_q=W_q, W_k=W_k, W_v=W_v, W_o=W_o))
    total = np.zeros((B, DM), dtype=np.float32)
    for c in range(N_CORES):
        total += res.results[c]["out"]
    return total.reshape(B, 1, DM)


# revision 4
# speedup vs baseline: 1.5375x; 1.0271x over previous
"""CachedAttention decode kernel for 8 TRN2 NeuronCores.

Problem: single-position cached attention (decode step).
  x:[16,1,2048], cache_k/v:[16,16,4096,128], W_q/k/v/o:[2048,2048] (torch
  Linear convention: y = x @ W.T).

Sharding: head-parallel across 8 cores, 2 heads/core. W_q/W_k/W_v
column-parallel (each core projects only its heads), W_o row-parallel
(each core computes a partial [16,2048] output; host sums the 8 partials).

Per-core device algorithm (all 16 batches, 2 heads):
  - projections computed TRANSPOSED: qT = Wq_rows @ x^T -> [d, b] per head,
    so q lands with head_dim on partitions (no on-chip transposes anywhere).
  - K cache is staged host-side as K^T [d, s] per (h,b); QK matmul uses
    K^T-tile as the stationary operand, q column as moving -> scores land
    [s_tile, 1] in PSUM with s on partitions.
  - softmax without max-subtraction (scores ~ N(0,1), exp is safe), exp on
    the scalar engine with the 1/sqrt(D) scale folded in.
  - PV: V natural [s, d] tiles stationary, p column moving, accumulated in
    PSUM -> context [d, b] per head.
  - the appended new position (k,v of the current token) is folded in as a
    rank-1 update batched over all (h,b) via a ones-broadcast matmul.
  - W_o partial: lhsT = normalized context [d, b], rhs = W_o^T slice.

Precision plan: KV cache stored as fp8 e3m4 (float8e3) scaled by 2.0 so
N(0,1) data fills the format's normal range (max 15.5) -- halves HBM
traffic vs bf16 again. The x2 score scale is folded into the exp scale
(SCALE/2); the x2 on V is cancelled by scaling W_o by 0.5 host-side (and
W_v by 2 so the new-position term matches). Everything else (x, weights,
q, p) rides fp16 (better mantissa than bf16, same speed); scores/psum
stay fp32.
"""
import sys

sys.path.insert(0, "/opt/trn_rl_repo")

from contextlib import ExitStack

import numpy as np

import concourse.bass as bass
import concourse.tile as tile
from concourse import bacc, mybir
from concourse.bass_utils import run_bass_kernel_spmd

# ---- problem constants (hardcoded; kernel.py must be self-contained) ----
B = 16          # batch
H = 16          # total heads
S = 4096        # cached sequence length
D = 128         # head dim
DM = 2048       # d_model
N_CORES = 8
HPC = H // N_CORES   # heads per core = 2
G = HPC * B          # (head, batch) pairs per core = 32
ST = S // 128        # s-tiles per (h,b) = 32
CH = 4               # max batches per KV DMA chunk
NG = B // CH         # chunk groups per head (host-packed contiguous)
KT = 16              # k-tiles over d_model contraction
SCALE = float(D) ** -0.5

F32 = mybir.dt.float32
DT_C = mybir.dt.float8e3     # KV cache storage (e3m4, scaled by KV_SCALE)
DT_A = mybir.dt.float16      # activations / weights / p
KV_SCALE = 2.0


def _build_kernel():
    nc = bacc.Bacc("TRN2", target_bir_lowering=False, debug=False)

    # DRAM parameters (per-core shards, host-prepared layouts)
    kt_d = nc.declare_dram_parameter("kt", [HPC, NG, 128, CH * S], DT_C, isOutput=False)
    vv_d = nc.declare_dram_parameter("vv", [HPC, NG, 128, CH * S], DT_C, isOutput=False)
    wq_d = nc.declare_dram_parameter("wq", [128, KT * HPC * D], DT_A, isOutput=False)
    wk_d = nc.declare_dram_parameter("wk", [128, KT * HPC * D], DT_A, isOutput=False)
    wv_d = nc.declare_dram_parameter("wv", [128, KT * HPC * D], DT_A, isOutput=False)
    wo_d = nc.declare_dram_parameter("wo", [128, HPC * DM], DT_A, isOutput=False)
    xt_d = nc.declare_dram_parameter("xt", [128, KT * B], DT_A, isOutput=False)
    out_d = nc.declare_dram_parameter("out", [B, DM], F32, isOutput=True)

    with tile.TileContext(nc) as tc, ExitStack() as ctx:
        wpool = ctx.enter_context(tc.tile_pool(name="w", bufs=1))
        spool = ctx.enter_context(tc.tile_pool(name="s", bufs=1))
        kpool = ctx.enter_context(tc.tile_pool(name="k", bufs=3))
        vpool = ctx.enter_context(tc.tile_pool(name="v", bufs=3))
        ppool = ctx.enter_context(tc.tile_pool(name="p", bufs=2 * CH))
        epool = ctx.enter_context(tc.tile_pool(name="e", bufs=2))
        ps_sc = ctx.enter_context(tc.tile_pool(name="psc", bufs=2, space="PSUM"))
        ps_cx = ctx.enter_context(tc.tile_pool(name="pcx", bufs=2, space="PSUM"))
        ps_ms = ctx.enter_context(tc.tile_pool(name="pms", bufs=2, space="PSUM"))
        ps_wo = ctx.enter_context(tc.tile_pool(name="pwo", bufs=2, space="PSUM"))

        # resident weights / activations. They ride the HWDGE rings AHEAD of
        # the KV stream at full rate (SWDGE crawls at packet round-robin
        # fairness and starved the PE of wk/wv for ~40us in v1): xt+wq on the
        # sync ring before the K chunks (~3us), wk/wv/wo on the scalar ring
        # before the V chunks (~9us; the first PV needs V chunk0 only after
        # chunk0's QK+exp, so this hides completely).
        xt_sb = wpool.tile([128, KT * B], DT_A, tag="xt")
        nc.sync.dma_start(xt_sb[:], xt_d[:])
        wq_sb = wpool.tile([128, KT * HPC * D], DT_A, tag="wq")
        nc.sync.dma_start(wq_sb[:], wq_d[:])
        wk_sb = wpool.tile([128, KT * HPC * D], DT_A, tag="wk")
        nc.scalar.dma_start(wk_sb[:], wk_d[:])
        wv_sb = wpool.tile([128, KT * HPC * D], DT_A, tag="wv")
        nc.scalar.dma_start(wv_sb[:], wv_d[:])
        wo_sb = wpool.tile([128, HPC * DM], DT_A, tag="wo")
        nc.scalar.dma_start(wo_sb[:], wo_d[:])

        ones_a = spool.tile([128, 1], DT_A, tag="ones_a")
        nc.vector.memset(ones_a[:], 1.0)

        q_a = spool.tile([128, G], DT_A, tag="q_a")
        knew_a = spool.tile([128, G], DT_A, tag="knew_a")
        vnewT = spool.tile([128, G], F32, tag="vnewT")
        p_new = spool.tile([1, G], F32, tag="p_new")
        denom = spool.tile([1, G], F32, tag="denom")
        dtot = spool.tile([1, G], F32, tag="dtot")
        recip = spool.tile([1, G], F32, tag="recip")
        out_sb = spool.tile([B, DM], F32, tag="out_sb")

        # ---- projections, transposed: proj[d, b] per head ----
        def emit_proj(w_sb, dst):
            for h in range(HPC):
                pr_ps = ps_ms.tile([128, B], F32, tag="misc", name=f"pr_{h}")
                for kk in range(KT):
                    nc.tensor.matmul(
                        pr_ps[:],
                        w_sb[:, kk * HPC * D + h * D: kk * HPC * D + (h + 1) * D],
                        xt_sb[:, kk * B: (kk + 1) * B],
                        start=(kk == 0), stop=(kk == KT - 1),
                    )
                nc.scalar.copy(dst[:, h * B: (h + 1) * B], pr_ps[:])

        # q-projection gates the very first QK matmuls -- emit it first.
        emit_proj(wq_sb, q_a)

        def emit_kv_proj_and_snew():
            emit_proj(wk_sb, knew_a)
            emit_proj(wv_sb, vnewT)
            sn_ps = ps_ms.tile([1, G], F32, tag="misc")
            for g in range(G):
                nc.tensor.matmul(
                    sn_ps[:, g: g + 1],
                    knew_a[:, g: g + 1],
                    q_a[:, g: g + 1],
                    start=True, stop=True,
                )
            nc.scalar.activation(p_new[:], sn_ps[:],
                                 mybir.ActivationFunctionType.Exp, scale=SCALE)

        # ---- main attention loop, software-pipelined by one chunk ----
        ctx_tiles = {}

        def emit_pv(ph, b0, cc, pv_sb, plist):
            ctx_ps = ctx_tiles[ph]
            for bl in range(cc):
                b = b0 + bl
                for si in range(ST):
                    nc.tensor.matmul(
                        ctx_ps[:, b: b + 1],
                        pv_sb[:, bl * S + si * 128: bl * S + (si + 1) * 128],
                        plist[bl][:, si: si + 1],
                        start=(si == 0), stop=(si == ST - 1),
                    )
            for bl in range(cc):
                g = ph * B + b0 + bl
                dn_ps = ps_ms.tile([1, ST], F32, tag="misc")
                nc.tensor.matmul(dn_ps[:], ones_a[:], plist[bl][:],
                                 start=True, stop=True)
                nc.vector.reduce_sum(denom[:, g: g + 1], dn_ps[:],
                                     axis=mybir.AxisListType.X)

        def emit_epilogue_pre(h):
            # Everything except the W_o matmuls -- ACT/GpSimd/DVE only.
            ctx_ps = ctx_tiles[h]
            hs = slice(h * B, (h + 1) * B)
            ctx_sb = epool.tile([128, B], F32, tag="ctx_sb")
            nc.scalar.copy(ctx_sb[:], ctx_ps[:])
            # + p_new * v_new  (vnewT carries the x2 KV scale via W_v)
            pb_bc = epool.tile([128, B], F32, tag="pb_bc")
            nc.gpsimd.partition_broadcast(pb_bc[:], p_new[:, hs])
            nt = epool.tile([128, B], F32, tag="nt")
            nc.vector.tensor_mul(nt[:], vnewT[:, hs], pb_bc[:])
            nc.vector.tensor_add(ctx_sb[:], ctx_sb[:], nt[:])
            # normalize by (denom + p_new); the x2 on ctx cancels via W_o*0.5
            nc.vector.tensor_add(dtot[:, hs], denom[:, hs], p_new[:, hs])
            nc.vector.reciprocal(recip[:, hs], dtot[:, hs])
            rb_bc = epool.tile([128, B], F32, tag="rb_bc")
            nc.gpsimd.partition_broadcast(rb_bc[:], recip[:, hs])
            ctx_n = epool.tile([128, B], DT_A, tag=f"ctx_n{h}", name=f"ctx_n{h}")
            nc.vector.tensor_mul(ctx_n[:], ctx_sb[:], rb_bc[:])
            return ctx_n

        def emit_epilogue_wo(h, ctx_n):
            for nchk in range(DM // 512):
                wo_ps = ps_wo.tile([B, 512], F32, tag="wo")
                nc.tensor.matmul(
                    wo_ps[:],
                    ctx_n[:],
                    wo_sb[:, h * DM + nchk * 512: h * DM + (nchk + 1) * 512],
                    start=True, stop=True,
                )
                if h == 0:
                    nc.scalar.copy(out_sb[:, nchk * 512: (nchk + 1) * 512], wo_ps[:])
                else:
                    nc.vector.tensor_add(out_sb[:, nchk * 512: (nchk + 1) * 512],
                                         out_sb[:, nchk * 512: (nchk + 1) * 512],
                                         wo_ps[:])

        pend = None
        wo_pend = None
        idx = 0
        for h in range(HPC):
            ctx_tiles[h] = ps_cx.tile([128, B], F32, tag="ctx", name=f"ctx_{h}")
            b0 = 0
            for ng in range(NG):
                cc = CH
                kt_sb = kpool.tile([128, CH * S], DT_C, tag="kt")
                nc.sync.dma_start(kt_sb[:], kt_d[h, ng])
                v_sb = vpool.tile([128, CH * S], DT_C, tag="vt")
                nc.scalar.dma_start(v_sb[:], vv_d[h, ng])
                plist = []
                for bl in range(cc):
                    g = h * B + b0 + bl
                    sc_ps = ps_sc.tile([128, ST], F32, tag="sc")
                    for si in range(ST):
                        nc.tensor.matmul(
                            sc_ps[:, si: si + 1],
                            kt_sb[:, bl * S + si * 128: bl * S + (si + 1) * 128],
                            q_a[:, g: g + 1],
                            start=True, stop=True,
                        )
                    p_sb = ppool.tile([128, ST], DT_A, tag="p")
                    # scores carry the x{KV_SCALE} from the stored K
                    nc.scalar.activation(p_sb[:], sc_ps[:],
                                         mybir.ActivationFunctionType.Exp,
                                         scale=SCALE / KV_SCALE)
                    plist.append(p_sb)
                if pend is not None:
                    emit_pv(*pend)
                    if wo_pend is not None:
                        emit_epilogue_wo(*wo_pend)
                        wo_pend = None
                    if pend[0] != h:
                        wo_pend = (pend[0], emit_epilogue_pre(pend[0]))
                pend = (h, b0, cc, v_sb, plist)
                if idx == 2:
                    emit_kv_proj_and_snew()
                idx += 1
                b0 += cc
        emit_pv(*pend)
        wo_pend2 = (HPC - 1, emit_epilogue_pre(HPC - 1))
        if wo_pend is not None:
            emit_epilogue_wo(*wo_pend)
        emit_epilogue_wo(*wo_pend2)

        nc.sync.dma_start(out_d[:], out_sb[:])

    nc.finalize()
    return nc


_NC_CACHE = None


def _get_kernel():
    global _NC_CACHE
    if _NC_CACHE is None:
        _NC_CACHE = _build_kernel()
    return _NC_CACHE


def _np_c(a):
    # KV cache quantization: scale into e3m4's normal range, clip for safety
    return np.clip(a * KV_SCALE, -15.0, 15.0).astype(mybir.dt.np(DT_C))


def _np_a(a):
    return np.ascontiguousarray(a, dtype=mybir.dt.np(DT_A))


def _shard_inputs(x, cache_k, cache_v, W_q, W_k, W_v, W_o):
    """Build per-core input maps with the on-device layouts."""
    x = np.asarray(x, dtype=np.float32)
    cache_k = np.asarray(cache_k, dtype=np.float32)
    cache_v = np.asarray(cache_v, dtype=np.float32)
    W_q = np.asarray(W_q, dtype=np.float32)
    W_k = np.asarray(W_k, dtype=np.float32)
    # fold the KV_SCALE bookkeeping into the projection weights:
    #   vnew must carry the same x2 as the stored V cache -> W_v * 2
    #   the x2 on the whole context is cancelled at the end -> W_o * 0.5
    W_v = np.asarray(W_v, dtype=np.float32) * KV_SCALE
    W_o = np.asarray(W_o, dtype=np.float32) * (1.0 / KV_SCALE)

    # xt[p, kk*B + b] = x[b, 0, kk*128 + p]  (shared by all cores)
    xt = _np_a(
        x[:, 0, :].T.reshape(KT, 128, B).transpose(1, 0, 2).reshape(128, KT * B)
    )

    in_maps = []
    for c in range(N_CORES):
        rows = slice(c * HPC * D, (c + 1) * HPC * D)
        # K^T per (h,b): [d, s]; pack CH batches along free dim per chunk
        k_c = cache_k[:, c * HPC:(c + 1) * HPC]          # [B, HPC, S, D]
        k_t = k_c.transpose(1, 0, 3, 2)                  # [HPC, B, D, S]
        k_t = k_t.reshape(HPC, NG, CH, 128, S).transpose(0, 1, 3, 2, 4)
        k_t = k_t.reshape(HPC, NG, 128, CH * S)
        # V natural per (h,b): rows s in tiles of 128 on partitions:
        # v[h, b, p, si*128 + d] = V[si*128 + p, d]
        v_c = cache_v[:, c * HPC:(c + 1) * HPC]          # [B, HPC, S, D]
        v_t = v_c.transpose(1, 0, 2, 3)                  # [HPC, B, S, D]
        v_t = v_t.reshape(HPC, B, ST, 128, D).transpose(0, 1, 3, 2, 4)
        v_t = v_t.reshape(HPC, NG, CH, 128, ST * D).transpose(0, 1, 3, 2, 4)
        v_t = v_t.reshape(HPC, NG, 128, CH * S)

        def wslice(W):
            # w[p, kk*HPC*D + h*D + m] = W[rows][h*D + m, kk*128 + p]
            wr = W[rows, :]                              # [HPC*D, DM]
            wr = wr.reshape(HPC * D, KT, 128).transpose(2, 1, 0)   # [p, kk, m]
            return _np_a(wr.reshape(128, KT * HPC * D))

        # wo[p, h*DM + j] = W_o[j, c*HPC*D + h*128 + p]
        wo = W_o[:, rows].T.reshape(HPC, 128, DM).transpose(1, 0, 2)
        wo = _np_a(wo.reshape(128, HPC * DM))

        in_maps.append({
            "kt": _np_c(k_t),
            "vv": _np_c(v_t),
            "wq": wslice(W_q),
            "wk": wslice(W_k),
            "wv": wslice(W_v),
            "wo": wo,
            "xt": xt,
        })
    return in_maps


def run_sharded(inputs, trace=False):
    """Run the SPMD kernel; returns BassKernelResults."""
    nc = _get_kernel()
    in_maps = _shard_inputs(**inputs)
    res = run_bass_kernel_spmd(nc, in_maps, core_ids=list(range(N_CORES)),
                               trace=trace)
    return res


def kernel(x, cache_k, cache_v, W_q, W_k, W_v, W_o) -> np.ndarray:
    res = run_sharded(dict(x=x, cache_k=cache_k, cache_v=cache_v,
                           W_q=W_q, W_k=W_k, W_v=W_v, W_o=W_o))
    total = np.zeros((B, DM), dtype=np.float32)
    for c in range(N_CORES):
        total += res.results[c]["out"]
    return total.reshape(B, 1, DM)


# revision 7
# speedup vs baseline: 1.6356x; 1.0638x over previous
"""CachedAttention decode kernel for 8 TRN2 NeuronCores.

Problem: single-position cached attention (decode step).
  x:[16,1,2048], cache_k/v:[16,16,4096,128], W_q/k/v/o:[2048,2048] (torch
  Linear convention: y = x @ W.T).

Sharding: head-parallel across 8 cores, 2 heads/core. W_q/W_k/W_v
column-parallel (each core projects only its heads), W_o row-parallel
(each core computes a partial [16,2048] output; host sums the 8 partials).

Per-core device algorithm (all 16 batches, 2 heads):
  - projections computed TRANSPOSED: qT = Wq_rows @ x^T -> [d, b] per head,
    so q lands with head_dim on partitions (no on-chip transposes anywhere).
  - K cache is staged host-side as K^T [d, s] per (h,b); QK matmul uses
    K^T-tile as the stationary operand, q column as moving -> scores land
    [s_tile, 1] in PSUM with s on partitions.
  - softmax without max-subtraction (scores ~ N(0,1), exp is safe), exp on
    the scalar engine with the 1/sqrt(D) scale folded in.
  - PV: V natural [s, d] tiles stationary, p column moving, accumulated in
    PSUM -> context [d, b] per head.
  - the appended new position (k,v of the current token) is folded in as a
    rank-1 update batched over all (h,b) via a ones-broadcast matmul.
  - W_o partial: lhsT = normalized context [d, b], rhs = W_o^T slice.

Precision plan: KV cache stored as fp8 e3m4 (float8e3) scaled by 2.0 so
N(0,1) data fills the format's normal range (max 15.5) -- halves HBM
traffic vs bf16 again. The x2 score scale is folded into the exp scale
(SCALE/2); the x2 on V is cancelled by scaling W_o by 0.5 host-side (and
W_v by 2 so the new-position term matches). Everything else (x, weights,
q, p) rides fp16 (better mantissa than bf16, same speed); scores/psum
stay fp32.
"""
import sys

sys.path.insert(0, "/opt/trn_rl_repo")

from contextlib import ExitStack

import numpy as np

import concourse.bass as bass
import concourse.tile as tile
from concourse import bacc, mybir
from concourse.bass_utils import run_bass_kernel_spmd

# ---- problem constants (hardcoded; kernel.py must be self-contained) ----
B = 16          # batch
H = 16          # total heads
S = 4096        # cached sequence length
D = 128         # head dim
DM = 2048       # d_model
N_CORES = 8
HPC = H // N_CORES   # heads per core = 2
G = HPC * B          # (head, batch) pairs per core = 32
ST = S // 128        # s-tiles per (h,b) = 32
CH = 2               # max batches per KV DMA chunk (1 MiB fp8 transfers)
NG = B // CH         # chunk groups per head (host-packed contiguous)
KT = 16              # k-tiles over d_model contraction
SCALE = float(D) ** -0.5

F32 = mybir.dt.float32
DT_C = mybir.dt.float8e3     # KV cache storage (e3m4, scaled by KV_SCALE)
DT_A = mybir.dt.float16      # activations / weights / p
KV_SCALE = 2.0


def _build_kernel():
    nc = bacc.Bacc("TRN2", target_bir_lowering=False, debug=False)

    # DRAM parameters (per-core shards, host-prepared layouts)
    kt_d = nc.declare_dram_parameter("kt", [HPC, NG, 128, CH * S], DT_C, isOutput=False)
    vv_d = nc.declare_dram_parameter("vv", [HPC, NG, 128, CH * S], DT_C, isOutput=False)
    wq_d = nc.declare_dram_parameter("wq", [128, KT * HPC * D], DT_A, isOutput=False)
    wk_d = nc.declare_dram_parameter("wk", [128, KT * HPC * D], DT_A, isOutput=False)
    wv_d = nc.declare_dram_parameter("wv", [128, KT * HPC * D], DT_A, isOutput=False)
    wo_d = nc.declare_dram_parameter("wo", [128, HPC * DM], DT_A, isOutput=False)
    xt_d = nc.declare_dram_parameter("xt", [128, KT * B], DT_A, isOutput=False)
    out_d = nc.declare_dram_parameter("out", [B, DM], F32, isOutput=True)

    with tile.TileContext(nc) as tc, ExitStack() as ctx:
        wpool = ctx.enter_context(tc.tile_pool(name="w", bufs=1))
        spool = ctx.enter_context(tc.tile_pool(name="s", bufs=1))
        kpool = ctx.enter_context(tc.tile_pool(name="k", bufs=5))
        vpool = ctx.enter_context(tc.tile_pool(name="v", bufs=5))
        ppool = ctx.enter_context(tc.tile_pool(name="p", bufs=2 * CH))
        epool = ctx.enter_context(tc.tile_pool(name="e", bufs=2))
        ps_sc = ctx.enter_context(tc.tile_pool(name="psc", bufs=2, space="PSUM"))
        ps_cx = ctx.enter_context(tc.tile_pool(name="pcx", bufs=2, space="PSUM"))
        ps_ms = ctx.enter_context(tc.tile_pool(name="pms", bufs=2, space="PSUM"))
        ps_wo = ctx.enter_context(tc.tile_pool(name="pwo", bufs=2, space="PSUM"))

        # resident weights / activations. They ride the HWDGE rings AHEAD of
        # the KV stream at full rate (SWDGE crawls at packet round-robin
        # fairness and starved the PE of wk/wv for ~40us in v1): xt+wq on the
        # sync ring before the K chunks (~3us), wk/wv/wo on the scalar ring
        # before the V chunks (~9us; the first PV needs V chunk0 only after
        # chunk0's QK+exp, so this hides completely).
        xt_sb = wpool.tile([128, KT * B], DT_A, tag="xt")
        nc.sync.dma_start(xt_sb[:], xt_d[:])
        wq_sb = wpool.tile([128, KT * HPC * D], DT_A, tag="wq")
        nc.sync.dma_start(wq_sb[:], wq_d[:])
        wk_sb = wpool.tile([128, KT * HPC * D], DT_A, tag="wk")
        nc.scalar.dma_start(wk_sb[:], wk_d[:])
        wv_sb = wpool.tile([128, KT * HPC * D], DT_A, tag="wv")
        nc.scalar.dma_start(wv_sb[:], wv_d[:])
        wo_sb = wpool.tile([128, HPC * DM], DT_A, tag="wo")
        nc.scalar.dma_start(wo_sb[:], wo_d[:])

        ones_a = spool.tile([128, 1], DT_A, tag="ones_a")
        nc.vector.memset(ones_a[:], 1.0)

        q_a = spool.tile([128, G], DT_A, tag="q_a")
        knew_a = spool.tile([128, G], DT_A, tag="knew_a")
        vnewT = spool.tile([128, G], F32, tag="vnewT")
        p_new = spool.tile([1, G], F32, tag="p_new")
        denom = spool.tile([1, G], F32, tag="denom")
        dtot = spool.tile([1, G], F32, tag="dtot")
        recip = spool.tile([1, G], F32, tag="recip")
        out_sb = spool.tile([B, DM], F32, tag="out_sb")

        # ---- projections, transposed: proj[d, b] per head ----
        def emit_proj(w_sb, dst):
            for h in range(HPC):
                pr_ps = ps_ms.tile([128, B], F32, tag="misc", name=f"pr_{h}")
                for kk in range(KT):
                    nc.tensor.matmul(
                        pr_ps[:],
                        w_sb[:, kk * HPC * D + h * D: kk * HPC * D + (h + 1) * D],
                        xt_sb[:, kk * B: (kk + 1) * B],
                        start=(kk == 0), stop=(kk == KT - 1),
                    )
                nc.scalar.copy(dst[:, h * B: (h + 1) * B], pr_ps[:])

        # q-projection gates the very first QK matmuls -- emit it first.
        emit_proj(wq_sb, q_a)

        def emit_kv_proj_and_snew():
            emit_proj(wk_sb, knew_a)
            emit_proj(wv_sb, vnewT)
            sn_ps = ps_ms.tile([1, G], F32, tag="misc")
            for g in range(G):
                nc.tensor.matmul(
                    sn_ps[:, g: g + 1],
                    knew_a[:, g: g + 1],
                    q_a[:, g: g + 1],
                    start=True, stop=True,
                )
            nc.scalar.activation(p_new[:], sn_ps[:],
                                 mybir.ActivationFunctionType.Exp, scale=SCALE)

        # ---- main attention loop, software-pipelined by one chunk ----
        ctx_tiles = {}

        def emit_pv(ph, b0, cc, pv_sb, plist):
            ctx_ps = ctx_tiles[ph]
            for bl in range(cc):
                b = b0 + bl
                for si in range(ST):
                    nc.tensor.matmul(
                        ctx_ps[:, b: b + 1],
                        pv_sb[:, bl * S + si * 128: bl * S + (si + 1) * 128],
                        plist[bl][:, si: si + 1],
                        start=(si == 0), stop=(si == ST - 1),
                    )
            for bl in range(cc):
                g = ph * B + b0 + bl
                dn_ps = ps_ms.tile([1, ST], F32, tag="misc")
                nc.tensor.matmul(dn_ps[:], ones_a[:], plist[bl][:],
                                 start=True, stop=True)
                nc.vector.reduce_sum(denom[:, g: g + 1], dn_ps[:],
                                     axis=mybir.AxisListType.X)

        def emit_epilogue_pre(h):
            # Everything except the W_o matmuls -- ACT/GpSimd/DVE only.
            ctx_ps = ctx_tiles[h]
            hs = slice(h * B, (h + 1) * B)
            ctx_sb = epool.tile([128, B], F32, tag="ctx_sb")
            nc.scalar.copy(ctx_sb[:], ctx_ps[:])
            # + p_new * v_new  (vnewT carries the x2 KV scale via W_v)
            pb_bc = epool.tile([128, B], F32, tag="pb_bc")
            nc.gpsimd.partition_broadcast(pb_bc[:], p_new[:, hs])
            nt = epool.tile([128, B], F32, tag="nt")
            nc.vector.tensor_mul(nt[:], vnewT[:, hs], pb_bc[:])
            nc.vector.tensor_add(ctx_sb[:], ctx_sb[:], nt[:])
            # normalize by (denom + p_new); the x2 on ctx cancels via W_o*0.5
            nc.vector.tensor_add(dtot[:, hs], denom[:, hs], p_new[:, hs])
            nc.vector.reciprocal(recip[:, hs], dtot[:, hs])
            rb_bc = epool.tile([128, B], F32, tag="rb_bc")
            nc.gpsimd.partition_broadcast(rb_bc[:], recip[:, hs])
            ctx_n = epool.tile([128, B], DT_A, tag=f"ctx_n{h}", name=f"ctx_n{h}")
            nc.vector.tensor_mul(ctx_n[:], ctx_sb[:], rb_bc[:])
            return ctx_n

        def emit_epilogue_wo(h, ctx_n):
            for nchk in range(DM // 512):
                wo_ps = ps_wo.tile([B, 512], F32, tag="wo")
                nc.tensor.matmul(
                    wo_ps[:],
                    ctx_n[:],
                    wo_sb[:, h * DM + nchk * 512: h * DM + (nchk + 1) * 512],
                    start=True, stop=True,
                )
                if h == 0:
                    nc.scalar.copy(out_sb[:, nchk * 512: (nchk + 1) * 512], wo_ps[:])
                else:
                    nc.vector.tensor_add(out_sb[:, nchk * 512: (nchk + 1) * 512],
                                         out_sb[:, nchk * 512: (nchk + 1) * 512],
                                         wo_ps[:])

        pend = None
        wo_pend = None
        idx = 0
        for h in range(HPC):
            ctx_tiles[h] = ps_cx.tile([128, B], F32, tag="ctx", name=f"ctx_{h}")
            b0 = 0
            for ng in range(NG):
                cc = CH
                kt_sb = kpool.tile([128, CH * S], DT_C, tag="kt")
                nc.sync.dma_start(kt_sb[:], kt_d[h, ng])
                v_sb = vpool.tile([128, CH * S], DT_C, tag="vt")
                nc.scalar.dma_start(v_sb[:], vv_d[h, ng])
                plist = []
                for bl in range(cc):
                    g = h * B + b0 + bl
                    sc_ps = ps_sc.tile([128, ST], F32, tag="sc")
                    for si in range(ST):
                        nc.tensor.matmul(
                            sc_ps[:, si: si + 1],
                            kt_sb[:, bl * S + si * 128: bl * S + (si + 1) * 128],
                            q_a[:, g: g + 1],
                            start=True, stop=True,
                        )
                    p_sb = ppool.tile([128, ST], DT_A, tag="p")
                    # scores carry the x{KV_SCALE} from the stored K
                    nc.scalar.activation(p_sb[:], sc_ps[:],
                                         mybir.ActivationFunctionType.Exp,
                                         scale=SCALE / KV_SCALE)
                    plist.append(p_sb)
                if pend is not None:
                    emit_pv(*pend)
                    if wo_pend is not None:
                        emit_epilogue_wo(*wo_pend)
                        wo_pend = None
                    if pend[0] != h:
                        wo_pend = (pend[0], emit_epilogue_pre(pend[0]))
                pend = (h, b0, cc, v_sb, plist)
                if idx == 3:
                    emit_kv_proj_and_snew()
                idx += 1
                b0 += cc
        emit_pv(*pend)
        wo_pend2 = (HPC - 1, emit_epilogue_pre(HPC - 1))
        if wo_pend is not None:
            emit_epilogue_wo(*wo_pend)
        emit_epilogue_wo(*wo_pend2)

        nc.sync.dma_start(out_d[:], out_sb[:])

    nc.finalize()
    return nc


_NC_CACHE = None


def _get_kernel():
    global _NC_CACHE
    if _NC_CACHE is None:
        _NC_CACHE = _build_kernel()
    return _NC_CACHE


def _np_c(a):
    # KV cache quantization: scale into e3m4's normal range, clip for safety
    return np.clip(a * KV_SCALE, -15.0, 15.0).astype(mybir.dt.np(DT_C))


def _np_a(a):
    return np.ascontiguousarray(a, dtype=mybir.dt.np(DT_A))


def _shard_inputs(x, cache_k, cache_v, W_q, W_k, W_v, W_o):
    """Build per-core input maps with the on-device layouts."""
    x = np.asarray(x, dtype=np.float32)
    cache_k = np.asarray(cache_k, dtype=np.float32)
    cache_v = np.asarray(cache_v, dtype=np.float32)
    W_q = np.asarray(W_q, dtype=np.float32)
    W_k = np.asarray(W_k, dtype=np.float32)
    # fold the KV_SCALE bookkeeping into the projection weights:
    #   vnew must carry the same x2 as the stored V cache -> W_v * 2
    #   the x2 on the whole context is cancelled at the end -> W_o * 0.5
    W_v = np.asarray(W_v, dtype=np.float32) * KV_SCALE
    W_o = np.asarray(W_o, dtype=np.float32) * (1.0 / KV_SCALE)

    # xt[p, kk*B + b] = x[b, 0, kk*128 + p]  (shared by all cores)
    xt = _np_a(
        x[:, 0, :].T.reshape(KT, 128, B).transpose(1, 0, 2).reshape(128, KT * B)
    )

    in_maps = []
    for c in range(N_CORES):
        rows = slice(c * HPC * D, (c + 1) * HPC * D)
        # K^T per (h,b): [d, s]; pack CH batches along free dim per chunk
        k_c = cache_k[:, c * HPC:(c + 1) * HPC]          # [B, HPC, S, D]
        k_t = k_c.transpose(1, 0, 3, 2)                  # [HPC, B, D, S]
        k_t = k_t.reshape(HPC, NG, CH, 128, S).transpose(0, 1, 3, 2, 4)
        k_t = k_t.reshape(HPC, NG, 128, CH * S)
        # V natural per (h,b): rows s in tiles of 128 on partitions:
        # v[h, b, p, si*128 + d] = V[si*128 + p, d]
        v_c = cache_v[:, c * HPC:(c + 1) * HPC]          # [B, HPC, S, D]
        v_t = v_c.transpose(1, 0, 2, 3)                  # [HPC, B, S, D]
        v_t = v_t.reshape(HPC, B, ST, 128, D).transpose(0, 1, 3, 2, 4)
        v_t = v_t.reshape(HPC, NG, CH, 128, ST * D).transpose(0, 1, 3, 2, 4)
        v_t = v_t.reshape(HPC, NG, 128, CH * S)

        def wslice(W):
            # w[p, kk*HPC*D + h*D + m] = W[rows][h*D + m, kk*128 + p]
            wr = W[rows, :]                              # [HPC*D, DM]
            wr = wr.reshape(HPC * D, KT, 128).transpose(2, 1, 0)   # [p, kk, m]
            return _np_a(wr.reshape(128, KT * HPC * D))

        # wo[p, h*DM + j] = W_o[j, c*HPC*D + h*128 + p]
        wo = W_o[:, rows].T.reshape(HPC, 128, DM).transpose(1, 0, 2)
        wo = _np_a(wo.reshape(128, HPC * DM))

        in_maps.append({
            "kt": _np_c(k_t),
            "vv": _np_c(v_t),
            "wq": wslice(W_q),
            "wk": wslice(W_k),
            "wv": wslice(W_v),
            "wo": wo,
            "xt": xt,
        })
    return in_maps


def run_sharded(inputs, trace=False):
    """Run the SPMD kernel; returns BassKernelResults."""
    nc = _get_kernel()
    in_maps = _shard_inputs(**inputs)
    res = run_bass_kernel_spmd(nc, in_maps, core_ids=list(range(N_CORES)),
                               trace=trace)
    return res


def kernel(x, cache_k, cache_v, W_q, W_k, W_v, W_o) -> np.ndarray:
    res = run_sharded(dict(x=x, cache_k=cache_k, cache_v=cache_v,
                           W_q=W_q, W_k=W_k, W_v=W_v, W_o=W_o))
    total = np.zeros((B, DM), dtype=np.float32)
    for c in range(N_CORES):
        total += res.results[c]["out"]
    return total.reshape(B, 1, DM)


# revision 10
# speedup vs baseline: 1.6494x; 1.0084x over previous
"""CachedAttention decode kernel for 8 TRN2 NeuronCores.

Problem: single-position cached attention (decode step).
  x:[16,1,2048], cache_k/v:[16,16,4096,128], W_q/k/v/o:[2048,2048] (torch
  Linear convention: y = x @ W.T).

Sharding: head-parallel across 8 cores, 2 heads/core. W_q/W_k/W_v
column-parallel (each core projects only its heads), W_o row-parallel
(each core computes a partial [16,2048] output; host sums the 8 partials).

Per-core device algorithm (all 16 batches, 2 heads):
  - projections computed TRANSPOSED: qT = Wq_rows @ x^T -> [d, b] per head,
    so q lands with head_dim on partitions (no on-chip transposes anywhere).
  - K cache is staged host-side as K^T [d, s] per (h,b); QK matmul uses
    K^T-tile as the stationary operand, q column as moving -> scores land
    [s_tile, 1] in PSUM with s on partitions.
  - softmax without max-subtraction (scores ~ N(0,1), exp is safe), exp on
    the scalar engine with the 1/sqrt(D) scale folded in.
  - PV: V natural [s, d] tiles stationary, p column moving, accumulated in
    PSUM -> context [d, b] per head.
  - the appended new position (k,v of the current token) is folded in as a
    rank-1 update batched over all (h,b) via a ones-broadcast matmul.
  - W_o partial: lhsT = normalized context [d, b], rhs = W_o^T slice.

Precision plan: KV cache stored as fp8 e3m4 (float8e3) scaled by 2.0 so
N(0,1) data fills the format's normal range (max 15.5) -- halves HBM
traffic vs bf16 again. The x2 score scale is folded into the exp scale
(SCALE/2); the x2 on V is cancelled by scaling W_o by 0.5 host-side (and
W_v by 2 so the new-position term matches). Everything else (x, weights,
q, p) rides fp16 (better mantissa than bf16, same speed); scores/psum
stay fp32.
"""
import sys

sys.path.insert(0, "/opt/trn_rl_repo")

from contextlib import ExitStack

import numpy as np

import concourse.bass as bass
import concourse.tile as tile
from concourse import bacc, mybir
from concourse.bass_utils import run_bass_kernel_spmd

# ---- problem constants (hardcoded; kernel.py must be self-contained) ----
B = 16          # batch
H = 16          # total heads
S = 4096        # cached sequence length
D = 128         # head dim
DM = 2048       # d_model
N_CORES = 8
HPC = H // N_CORES   # heads per core = 2
G = HPC * B          # (head, batch) pairs per core = 32
ST = S // 128        # s-tiles per (h,b) = 32
CH = 2               # max batches per KV DMA chunk (1 MiB fp8 transfers)
NG = B // CH         # chunk groups per head (host-packed contiguous)
KT = 16              # k-tiles over d_model contraction
SCALE = float(D) ** -0.5

F32 = mybir.dt.float32
DT_C = mybir.dt.float8e3     # KV cache storage (e3m4, scaled by KV_SCALE)
DT_A = mybir.dt.float16      # activations / weights / p
KV_SCALE = 2.0


def _build_kernel():
    nc = bacc.Bacc("TRN2", target_bir_lowering=False, debug=False)

    # DRAM parameters (per-core shards, host-prepared layouts)
    kt_d = nc.declare_dram_parameter("kt", [HPC, NG, 128, CH * S], DT_C, isOutput=False)
    vv_d = nc.declare_dram_parameter("vv", [HPC, NG, 128, CH * S], DT_C, isOutput=False)
    wq_d = nc.declare_dram_parameter("wq", [128, KT * HPC * D], DT_A, isOutput=False)
    wk_d = nc.declare_dram_parameter("wk", [128, KT * HPC * D], DT_A, isOutput=False)
    wv_d = nc.declare_dram_parameter("wv", [128, KT * HPC * D], DT_A, isOutput=False)
    wo_d = nc.declare_dram_parameter("wo", [128, HPC * DM], DT_A, isOutput=False)
    xt_d = nc.declare_dram_parameter("xt", [128, KT * B], DT_A, isOutput=False)
    out_d = nc.declare_dram_parameter("out", [B, DM], F32, isOutput=True)

    with tile.TileContext(nc) as tc, ExitStack() as ctx:
        wpool = ctx.enter_context(tc.tile_pool(name="w", bufs=1))
        spool = ctx.enter_context(tc.tile_pool(name="s", bufs=1))
        kpool = ctx.enter_context(tc.tile_pool(name="k", bufs=8))
        vpool = ctx.enter_context(tc.tile_pool(name="v", bufs=8))
        ppool = ctx.enter_context(tc.tile_pool(name="p", bufs=2 * CH))
        epool = ctx.enter_context(tc.tile_pool(name="e", bufs=2))
        ps_sc = ctx.enter_context(tc.tile_pool(name="psc", bufs=2, space="PSUM"))
        ps_cx = ctx.enter_context(tc.tile_pool(name="pcx", bufs=2, space="PSUM"))
        ps_ms = ctx.enter_context(tc.tile_pool(name="pms", bufs=2, space="PSUM"))
        ps_wo = ctx.enter_context(tc.tile_pool(name="pwo", bufs=2, space="PSUM"))

        # resident weights / activations. They ride the HWDGE rings AHEAD of
        # the KV stream at full rate (SWDGE crawls at packet round-robin
        # fairness and starved the PE of wk/wv for ~40us in v1): xt+wq on the
        # sync ring before the K chunks (~3us), wk/wv/wo on the scalar ring
        # before the V chunks (~9us; the first PV needs V chunk0 only after
        # chunk0's QK+exp, so this hides completely).
        xt_sb = wpool.tile([128, KT * B], DT_A, tag="xt")
        nc.sync.dma_start(xt_sb[:], xt_d[:])
        wq_sb = wpool.tile([128, KT * HPC * D], DT_A, tag="wq")
        nc.sync.dma_start(wq_sb[:], wq_d[:])
        # wk/wv/wo are interleaved into the chunk streams below (after chunk
        # idx 1) so K0/V0 land first and both rings stay byte-balanced.
        wk_sb = wpool.tile([128, KT * HPC * D], DT_A, tag="wk")
        wv_sb = wpool.tile([128, KT * HPC * D], DT_A, tag="wv")
        wo_sb = wpool.tile([128, HPC * DM], DT_A, tag="wo")

        ones_a = spool.tile([128, 1], DT_A, tag="ones_a")
        nc.vector.memset(ones_a[:], 1.0)

        q_a = spool.tile([128, G], DT_A, tag="q_a")
        knew_a = spool.tile([128, G], DT_A, tag="knew_a")
        vnewT = spool.tile([128, G], F32, tag="vnewT")
        p_new = spool.tile([1, G], F32, tag="p_new")
        denom = spool.tile([1, G], F32, tag="denom")
        dtot = spool.tile([1, G], F32, tag="dtot")
        recip = spool.tile([1, G], F32, tag="recip")
        out_sb = spool.tile([B, DM], F32, tag="out_sb")

        # ---- projections, transposed: proj[d, b] per head ----
        def emit_proj(w_sb, dst):
            for h in range(HPC):
                pr_ps = ps_ms.tile([128, B], F32, tag="misc", name=f"pr_{h}")
                for kk in range(KT):
                    nc.tensor.matmul(
                        pr_ps[:],
                        w_sb[:, kk * HPC * D + h * D: kk * HPC * D + (h + 1) * D],
                        xt_sb[:, kk * B: (kk + 1) * B],
                        start=(kk == 0), stop=(kk == KT - 1),
                    )
                nc.scalar.copy(dst[:, h * B: (h + 1) * B], pr_ps[:])

        # q-projection gates the very first QK matmuls -- emit it first.
        emit_proj(wq_sb, q_a)

        def emit_kv_proj_and_snew():
            emit_proj(wk_sb, knew_a)
            emit_proj(wv_sb, vnewT)
            sn_ps = ps_ms.tile([1, G], F32, tag="misc")
            for g in range(G):
                nc.tensor.matmul(
                    sn_ps[:, g: g + 1],
                    knew_a[:, g: g + 1],
                    q_a[:, g: g + 1],
                    start=True, stop=True,
                )
            nc.scalar.activation(p_new[:], sn_ps[:],
                                 mybir.ActivationFunctionType.Exp, scale=SCALE)

        # ---- main attention loop, software-pipelined by one chunk ----
        ctx_tiles = {}

        def emit_pv(ph, b0, cc, pv_sb, plist):
            ctx_ps = ctx_tiles[ph]
            for bl in range(cc):
                b = b0 + bl
                for si in range(ST):
                    nc.tensor.matmul(
                        ctx_ps[:, b: b + 1],
                        pv_sb[:, bl * S + si * 128: bl * S + (si + 1) * 128],
                        plist[bl][:, si: si + 1],
                        start=(si == 0), stop=(si == ST - 1),
                    )
            for bl in range(cc):
                g = ph * B + b0 + bl
                dn_ps = ps_ms.tile([1, ST], F32, tag="misc")
                nc.tensor.matmul(dn_ps[:], ones_a[:], plist[bl][:],
                                 start=True, stop=True)
                nc.vector.reduce_sum(denom[:, g: g + 1], dn_ps[:],
                                     axis=mybir.AxisListType.X)

        def emit_epilogue_pre(h):
            # Everything except the W_o matmuls -- ACT/GpSimd/DVE only.
            ctx_ps = ctx_tiles[h]
            hs = slice(h * B, (h + 1) * B)
            ctx_sb = epool.tile([128, B], F32, tag="ctx_sb")
            nc.scalar.copy(ctx_sb[:], ctx_ps[:])
            # + p_new * v_new  (vnewT carries the x2 KV scale via W_v)
            pb_bc = epool.tile([128, B], F32, tag="pb_bc")
            nc.gpsimd.partition_broadcast(pb_bc[:], p_new[:, hs])
            nt = epool.tile([128, B], F32, tag="nt")
            nc.vector.tensor_mul(nt[:], vnewT[:, hs], pb_bc[:])
            nc.vector.tensor_add(ctx_sb[:], ctx_sb[:], nt[:])
            # normalize by (denom + p_new); the x2 on ctx cancels via W_o*0.5
            nc.vector.tensor_add(dtot[:, hs], denom[:, hs], p_new[:, hs])
            nc.vector.reciprocal(recip[:, hs], dtot[:, hs])
            rb_bc = epool.tile([128, B], F32, tag="rb_bc")
            nc.gpsimd.partition_broadcast(rb_bc[:], recip[:, hs])
            ctx_n = epool.tile([128, B], DT_A, tag=f"ctx_n{h}", name=f"ctx_n{h}")
            nc.vector.tensor_mul(ctx_n[:], ctx_sb[:], rb_bc[:])
            return ctx_n

        def emit_epilogue_wo(h, ctx_n):
            for nchk in range(DM // 512):
                wo_ps = ps_wo.tile([B, 512], F32, tag="wo")
                nc.tensor.matmul(
                    wo_ps[:],
                    ctx_n[:],
                    wo_sb[:, h * DM + nchk * 512: h * DM + (nchk + 1) * 512],
                    start=True, stop=True,
                )
                if h == 0:
                    nc.scalar.copy(out_sb[:, nchk * 512: (nchk + 1) * 512], wo_ps[:])
                else:
                    nc.vector.tensor_add(out_sb[:, nchk * 512: (nchk + 1) * 512],
                                         out_sb[:, nchk * 512: (nchk + 1) * 512],
                                         wo_ps[:])

        pend = None
        wo_pend = None
        idx = 0
        for h in range(HPC):
            ctx_tiles[h] = ps_cx.tile([128, B], F32, tag="ctx", name=f"ctx_{h}")
            b0 = 0
            for ng in range(NG):
                cc = CH
                kt_sb = kpool.tile([128, CH * S], DT_C, tag="kt")
                nc.sync.dma_start(kt_sb[:], kt_d[h, ng])
                v_sb = vpool.tile([128, CH * S], DT_C, tag="vt")
                nc.scalar.dma_start(v_sb[:], vv_d[h, ng])
                plist = []
                for bl in range(cc):
                    g = h * B + b0 + bl
                    sc_ps = ps_sc.tile([128, ST], F32, tag="sc")
                    for si in range(ST):
                        nc.tensor.matmul(
                            sc_ps[:, si: si + 1],
                            kt_sb[:, bl * S + si * 128: bl * S + (si + 1) * 128],
                            q_a[:, g: g + 1],
                            start=True, stop=True,
                        )
                    p_sb = ppool.tile([128, ST], DT_A, tag="p")
                    # scores carry the x{KV_SCALE} from the stored K
                    nc.scalar.activation(p_sb[:], sc_ps[:],
                                         mybir.ActivationFunctionType.Exp,
                                         scale=SCALE / KV_SCALE)
                    plist.append(p_sb)
                if pend is not None:
                    emit_pv(*pend)
                    if wo_pend is not None:
                        emit_epilogue_wo(*wo_pend)
                        wo_pend = None
                    if pend[0] != h:
                        wo_pend = (pend[0], emit_epilogue_pre(pend[0]))
                pend = (h, b0, cc, v_sb, plist)
                if idx == 1:
                    # weight loads ride behind chunk 1 on each ring:
                    # sync carries wk (+19.0MB total), scalar wv+wo (+19.0MB)
                    nc.sync.dma_start(wk_sb[:], wk_d[:])
                    nc.scalar.dma_start(wv_sb[:], wv_d[:])
                    nc.scalar.dma_start(wo_sb[:], wo_d[:])
                if idx == 3:
                    emit_kv_proj_and_snew()
                idx += 1
                b0 += cc
        emit_pv(*pend)
        wo_pend2 = (HPC - 1, emit_epilogue_pre(HPC - 1))
        if wo_pend is not None:
            emit_epilogue_wo(*wo_pend)
        emit_epilogue_wo(*wo_pend2)

        nc.sync.dma_start(out_d[:], out_sb[:])

    nc.finalize()
    return nc


_NC_CACHE = None


def _get_kernel():
    global _NC_CACHE
    if _NC_CACHE is None:
        _NC_CACHE = _build_kernel()
    return _NC_CACHE


def _np_c(a):
    # KV cache quantization: scale into e3m4's normal range, clip for safety
    return np.clip(a * KV_SCALE, -15.0, 15.0).astype(mybir.dt.np(DT_C))


def _np_a(a):
    return np.ascontiguousarray(a, dtype=mybir.dt.np(DT_A))


def _shard_inputs(x, cache_k, cache_v, W_q, W_k, W_v, W_o):
    """Build per-core input maps with the on-device layouts."""
    x = np.asarray(x, dtype=np.float32)
    cache_k = np.asarray(cache_k, dtype=np.float32)
    cache_v = np.asarray(cache_v, dtype=np.float32)
    W_q = np.asarray(W_q, dtype=np.float32)
    W_k = np.asarray(W_k, dtype=np.float32)
    # fold the KV_SCALE bookkeeping into the projection weights:
    #   vnew must carry the same x2 as the stored V cache -> W_v * 2
    #   the x2 on the whole context is cancelled at the end -> W_o * 0.5
    W_v = np.asarray(W_v, dtype=np.float32) * KV_SCALE
    W_o = np.asarray(W_o, dtype=np.float32) * (1.0 / KV_SCALE)

    # xt[p, kk*B + b] = x[b, 0, kk*128 + p]  (shared by all cores)
    xt = _np_a(
        x[:, 0, :].T.reshape(KT, 128, B).transpose(1, 0, 2).reshape(128, KT * B)
    )

    in_maps = []
    for c in range(N_CORES):
        rows = slice(c * HPC * D, (c + 1) * HPC * D)
        # K^T per (h,b): [d, s]; pack CH batches along free dim per chunk
        k_c = cache_k[:, c * HPC:(c + 1) * HPC]          # [B, HPC, S, D]
        k_t = k_c.transpose(1, 0, 3, 2)                  # [HPC, B, D, S]
        k_t = k_t.reshape(HPC, NG, CH, 128, S).transpose(0, 1, 3, 2, 4)
        k_t = k_t.reshape(HPC, NG, 128, CH * S)
        # V natural per (h,b): rows s in tiles of 128 on partitions:
        # v[h, b, p, si*128 + d] = V[si*128 + p, d]
        v_c = cache_v[:, c * HPC:(c + 1) * HPC]          # [B, HPC, S, D]
        v_t = v_c.transpose(1, 0, 2, 3)                  # [HPC, B, S, D]
        v_t = v_t.reshape(HPC, B, ST, 128, D).transpose(0, 1, 3, 2, 4)
        v_t = v_t.reshape(HPC, NG, CH, 128, ST * D).transpose(0, 1, 3, 2, 4)
        v_t = v_t.reshape(HPC, NG, 128, CH * S)

        def wslice(W):
            # w[p, kk*HPC*D + h*D + m] = W[rows][h*D + m, kk*128 + p]
            wr = W[rows, :]                              # [HPC*D, DM]
            wr = wr.reshape(HPC * D, KT, 128).transpose(2, 1, 0)   # [p, kk, m]
            return _np_a(wr.reshape(128, KT * HPC * D))

        # wo[p, h*DM + j] = W_o[j, c*HPC*D + h*128 + p]
        wo = W_o[:, rows].T.reshape(HPC, 128, DM).transpose(1, 0, 2)
        wo = _np_a(wo.reshape(128, HPC * DM))

        in_maps.append({
            "kt": _np_c(k_t),
            "vv": _np_c(v_t),
            "wq": wslice(W_q),
            "wk": wslice(W_k),
            "wv": wslice(W_v),
            "wo": wo,
            "xt": xt,
        })
    return in_maps


def run_sharded(inputs, trace=False):
    """Run the SPMD kernel; returns BassKernelResults."""
    nc = _get_kernel()
    in_maps = _shard_inputs(**inputs)
    res = run_bass_kernel_spmd(nc, in_maps, core_ids=list(range(N_CORES)),
                               trace=trace)
    return res


def kernel(x, cache_k, cache_v, W_q, W_k, W_v, W_o) -> np.ndarray:
    res = run_sharded(dict(x=x, cache_k=cache_k, cache_v=cache_v,
                           W_q=W_q, W_k=W_k, W_v=W_v, W_o=W_o))
    total = np.zeros((B, DM), dtype=np.float32)
    for c in range(N_CORES):
        total += res.results[c]["out"]
    return total.reshape(B, 1, DM)


# revision 12
# speedup vs baseline: 1.6768x; 1.0166x over previous
"""CachedAttention decode kernel for 8 TRN2 NeuronCores.

Problem: single-position cached attention (decode step).
  x:[16,1,2048], cache_k/v:[16,16,4096,128], W_q/k/v/o:[2048,2048] (torch
  Linear convention: y = x @ W.T).

Sharding: head-parallel across 8 cores, 2 heads/core. W_q/W_k/W_v
column-parallel (each core projects only its heads), W_o row-parallel
(each core computes a partial [16,2048] output; host sums the 8 partials).

Per-core device algorithm (all 16 batches, 2 heads):
  - projections computed TRANSPOSED: qT = Wq_rows @ x^T -> [d, b] per head,
    so q lands with head_dim on partitions (no on-chip transposes anywhere).
  - K cache is staged host-side as K^T [d, s] per (h,b); QK matmul uses
    K^T-tile as the stationary operand, q column as moving -> scores land
    [s_tile, 1] in PSUM with s on partitions.
  - softmax without max-subtraction (scores ~ N(0,1), exp is safe), exp on
    the scalar engine with the 1/sqrt(D) scale folded in.
  - PV: V natural [s, d] tiles stationary, p column moving, accumulated in
    PSUM -> context [d, b] per head.
  - the appended new position (k,v of the current token) is folded in as a
    rank-1 update batched over all (h,b) via a ones-broadcast matmul.
  - W_o partial: lhsT = normalized context [d, b], rhs = W_o^T slice.

Precision plan: KV cache stored as fp8 e3m4 (float8e3) scaled by 2.0 so
N(0,1) data fills the format's normal range (max 15.5) -- halves HBM
traffic vs bf16 again. The x2 score scale is folded into the exp scale
(SCALE/2); the x2 on V is cancelled by scaling W_o by 0.5 host-side (and
W_v by 2 so the new-position term matches). Everything else (x, weights,
q, p) rides fp16 (better mantissa than bf16, same speed); scores/psum
stay fp32.
"""
import sys

sys.path.insert(0, "/opt/trn_rl_repo")

from contextlib import ExitStack

import numpy as np

import concourse.bass as bass
import concourse.tile as tile
from concourse import bacc, mybir
from concourse.bass_utils import run_bass_kernel_spmd

# ---- problem constants (hardcoded; kernel.py must be self-contained) ----
B = 16          # batch
H = 16          # total heads
S = 4096        # cached sequence length
D = 128         # head dim
DM = 2048       # d_model
N_CORES = 8
HPC = H // N_CORES   # heads per core = 2
G = HPC * B          # (head, batch) pairs per core = 32
ST = S // 128        # s-tiles per (h,b) = 32
CH = 2               # max batches per KV DMA chunk (1 MiB fp8 transfers)
NG = B // CH         # chunk groups per head (host-packed contiguous)
KT = 16              # k-tiles over d_model contraction
SCALE = float(D) ** -0.5

F32 = mybir.dt.float32
DT_C = mybir.dt.float8e3     # KV cache storage (e3m4, scaled by KV_SCALE)
DT_A = mybir.dt.float16      # activations / weights / p
KV_SCALE = 2.0


def _build_kernel():
    nc = bacc.Bacc("TRN2", target_bir_lowering=False, debug=False)

    # DRAM parameters (per-core shards, host-prepared layouts)
    kt_d = nc.declare_dram_parameter("kt", [HPC, NG, 128, CH * S], DT_C, isOutput=False)
    vv_d = nc.declare_dram_parameter("vv", [HPC, NG, 128, CH * S], DT_C, isOutput=False)
    wq_d = nc.declare_dram_parameter("wq", [128, KT * HPC * D], DT_A, isOutput=False)
    wk_d = nc.declare_dram_parameter("wk", [128, KT * HPC * D], DT_A, isOutput=False)
    wv_d = nc.declare_dram_parameter("wv", [128, KT * HPC * D], DT_A, isOutput=False)
    wo_d = nc.declare_dram_parameter("wo", [128, HPC * DM], DT_A, isOutput=False)
    xt_d = nc.declare_dram_parameter("xt", [128, KT * B], DT_A, isOutput=False)
    out_d = nc.declare_dram_parameter("out", [B, DM], F32, isOutput=True)

    with tile.TileContext(nc) as tc, ExitStack() as ctx:
        wpool = ctx.enter_context(tc.tile_pool(name="w", bufs=1))
        spool = ctx.enter_context(tc.tile_pool(name="s", bufs=1))
        kpool = ctx.enter_context(tc.tile_pool(name="k", bufs=8))
        vpool = ctx.enter_context(tc.tile_pool(name="v", bufs=8))
        ppool = ctx.enter_context(tc.tile_pool(name="p", bufs=2 * CH))
        epool = ctx.enter_context(tc.tile_pool(name="e", bufs=2))
        ps_sc = ctx.enter_context(tc.tile_pool(name="psc", bufs=2, space="PSUM"))
        ps_cx = ctx.enter_context(tc.tile_pool(name="pcx", bufs=2, space="PSUM"))
        ps_ms = ctx.enter_context(tc.tile_pool(name="pms", bufs=2, space="PSUM"))
        ps_wo = ctx.enter_context(tc.tile_pool(name="pwo", bufs=2, space="PSUM"))

        # resident weights / activations. They ride the HWDGE rings AHEAD of
        # the KV stream at full rate (SWDGE crawls at packet round-robin
        # fairness and starved the PE of wk/wv for ~40us in v1): xt+wq on the
        # sync ring before the K chunks (~3us), wk/wv/wo on the scalar ring
        # before the V chunks (~9us; the first PV needs V chunk0 only after
        # chunk0's QK+exp, so this hides completely).
        xt_sb = wpool.tile([128, KT * B], DT_A, tag="xt")
        nc.sync.dma_start(xt_sb[:], xt_d[:])
        wq_sb = wpool.tile([128, KT * HPC * D], DT_A, tag="wq")
        nc.sync.dma_start(wq_sb[:], wq_d[:])
        # wk/wv/wo are interleaved into the chunk streams below (after chunk
        # idx 1) so K0/V0 land first and both rings stay byte-balanced.
        wk_sb = wpool.tile([128, KT * HPC * D], DT_A, tag="wk")
        wv_sb = wpool.tile([128, KT * HPC * D], DT_A, tag="wv")
        wo_sb = wpool.tile([128, HPC * DM], DT_A, tag="wo")

        ones_a = spool.tile([128, 1], DT_A, tag="ones_a")
        nc.vector.memset(ones_a[:], 1.0)

        q_a = spool.tile([128, G], DT_A, tag="q_a")
        knew_a = spool.tile([128, G], DT_A, tag="knew_a")
        vnewT = spool.tile([128, G], F32, tag="vnewT")
        p_new = spool.tile([1, G], F32, tag="p_new")
        denom = spool.tile([1, G], F32, tag="denom")
        dtot = spool.tile([1, G], F32, tag="dtot")
        recip = spool.tile([1, G], F32, tag="recip")
        out_sb = spool.tile([B, DM], F32, tag="out_sb")

        # ---- projections, transposed: proj[d, b] per head ----
        def emit_proj(w_sb, dst):
            for h in range(HPC):
                pr_ps = ps_ms.tile([128, B], F32, tag="misc", name=f"pr_{h}")
                for kk in range(KT):
                    nc.tensor.matmul(
                        pr_ps[:],
                        w_sb[:, kk * HPC * D + h * D: kk * HPC * D + (h + 1) * D],
                        xt_sb[:, kk * B: (kk + 1) * B],
                        start=(kk == 0), stop=(kk == KT - 1),
                    )
                nc.scalar.copy(dst[:, h * B: (h + 1) * B], pr_ps[:])

        # q-projection gates the very first QK matmuls -- emit it first.
        emit_proj(wq_sb, q_a)

        def emit_kv_proj_and_snew():
            emit_proj(wk_sb, knew_a)
            emit_proj(wv_sb, vnewT)
            sn_ps = ps_ms.tile([1, G], F32, tag="misc")
            for g in range(G):
                nc.tensor.matmul(
                    sn_ps[:, g: g + 1],
                    knew_a[:, g: g + 1],
                    q_a[:, g: g + 1],
                    start=True, stop=True,
                )
            nc.scalar.activation(p_new[:], sn_ps[:],
                                 mybir.ActivationFunctionType.Exp, scale=SCALE)

        # ---- main attention loop, software-pipelined by one chunk ----
        ctx_tiles = {}

        def emit_pv(ph, b0, cc, pv_sb, plist):
            ctx_ps = ctx_tiles[ph]
            for bl in range(cc):
                b = b0 + bl
                for si in range(ST):
                    nc.tensor.matmul(
                        ctx_ps[:, b: b + 1],
                        pv_sb[:, bl * S + si * 128: bl * S + (si + 1) * 128],
                        plist[bl][:, si: si + 1],
                        start=(si == 0), stop=(si == ST - 1),
                    )
            for bl in range(cc):
                g = ph * B + b0 + bl
                dn_ps = ps_ms.tile([1, ST], F32, tag="misc")
                nc.tensor.matmul(dn_ps[:], ones_a[:], plist[bl][:],
                                 start=True, stop=True)
                nc.vector.reduce_sum(denom[:, g: g + 1], dn_ps[:],
                                     axis=mybir.AxisListType.X)

        def emit_epilogue_pre(h):
            # Everything except the W_o matmuls -- ACT/GpSimd/DVE only.
            ctx_ps = ctx_tiles[h]
            hs = slice(h * B, (h + 1) * B)
            ctx_sb = epool.tile([128, B], F32, tag="ctx_sb")
            nc.scalar.copy(ctx_sb[:], ctx_ps[:])
            # + p_new * v_new  (vnewT carries the x2 KV scale via W_v)
            pb_bc = epool.tile([128, B], F32, tag="pb_bc")
            nc.gpsimd.partition_broadcast(pb_bc[:], p_new[:, hs])
            nt = epool.tile([128, B], F32, tag="nt")
            nc.vector.tensor_mul(nt[:], vnewT[:, hs], pb_bc[:])
            nc.vector.tensor_add(ctx_sb[:], ctx_sb[:], nt[:])
            # normalize by (denom + p_new); the x2 on ctx cancels via W_o*0.5
            nc.vector.tensor_add(dtot[:, hs], denom[:, hs], p_new[:, hs])
            nc.vector.reciprocal(recip[:, hs], dtot[:, hs])
            rb_bc = epool.tile([128, B], F32, tag="rb_bc")
            nc.gpsimd.partition_broadcast(rb_bc[:], recip[:, hs])
            ctx_n = epool.tile([128, B], DT_A, tag=f"ctx_n{h}", name=f"ctx_n{h}")
            nc.vector.tensor_mul(ctx_n[:], ctx_sb[:], rb_bc[:])
            return ctx_n

        def emit_epilogue_wo(h, ctx_n):
            for nchk in range(DM // 512):
                wo_ps = ps_wo.tile([B, 512], F32, tag="wo")
                nc.tensor.matmul(
                    wo_ps[:],
                    ctx_n[:],
                    wo_sb[:, h * DM + nchk * 512: h * DM + (nchk + 1) * 512],
                    start=True, stop=True,
                )
                if h == 0:
                    nc.scalar.copy(out_sb[:, nchk * 512: (nchk + 1) * 512], wo_ps[:])
                else:
                    nc.vector.tensor_add(out_sb[:, nchk * 512: (nchk + 1) * 512],
                                         out_sb[:, nchk * 512: (nchk + 1) * 512],
                                         wo_ps[:])

        pend = None
        wo_pend = None
        idx = 0
        for h in range(HPC):
            ctx_tiles[h] = ps_cx.tile([128, B], F32, tag="ctx", name=f"ctx_{h}")
            b0 = 0
            for ng in range(NG):
                cc = CH
                kt_sb = kpool.tile([128, CH * S], DT_C, tag="kt")
                nc.sync.dma_start(kt_sb[:], kt_d[h, ng])
                v_sb = vpool.tile([128, CH * S], DT_C, tag="vt")
                # V rides the SAME sync ring: issuing it from ACT puts the
                # dma_start behind exp instructions in ACT's strict FIFO, and
                # an exp waiting on PE scores blocks the V stream (v4 trace).
                nc.sync.dma_start(v_sb[:], vv_d[h, ng])
                plist = []
                for bl in range(cc):
                    g = h * B + b0 + bl
                    sc_ps = ps_sc.tile([128, ST], F32, tag="sc")
                    for si in range(ST):
                        nc.tensor.matmul(
                            sc_ps[:, si: si + 1],
                            kt_sb[:, bl * S + si * 128: bl * S + (si + 1) * 128],
                            q_a[:, g: g + 1],
                            start=True, stop=True,
                        )
                    p_sb = ppool.tile([128, ST], DT_A, tag="p")
                    # scores carry the x{KV_SCALE} from the stored K
                    nc.scalar.activation(p_sb[:], sc_ps[:],
                                         mybir.ActivationFunctionType.Exp,
                                         scale=SCALE / KV_SCALE)
                    plist.append(p_sb)
                if pend is not None:
                    emit_pv(*pend)
                    if wo_pend is not None:
                        emit_epilogue_wo(*wo_pend)
                        wo_pend = None
                    if pend[0] != h:
                        wo_pend = (pend[0], emit_epilogue_pre(pend[0]))
                pend = (h, b0, cc, v_sb, plist)
                if idx == 1:
                    # weight loads ride behind chunk 1 (needed by idx==3)
                    nc.sync.dma_start(wk_sb[:], wk_d[:])
                    nc.sync.dma_start(wv_sb[:], wv_d[:])
                    nc.sync.dma_start(wo_sb[:], wo_d[:])
                if idx == 3:
                    emit_kv_proj_and_snew()
                idx += 1
                b0 += cc
        emit_pv(*pend)
        wo_pend2 = (HPC - 1, emit_epilogue_pre(HPC - 1))
        if wo_pend is not None:
            emit_epilogue_wo(*wo_pend)
        emit_epilogue_wo(*wo_pend2)

        nc.sync.dma_start(out_d[:], out_sb[:])

    nc.finalize()
    return nc


_NC_CACHE = None


def _get_kernel():
    global _NC_CACHE
    if _NC_CACHE is None:
        _NC_CACHE = _build_kernel()
    return _NC_CACHE


def _np_c(a):
    # KV cache quantization: scale into e3m4's normal range, clip for safety
    return np.clip(a * KV_SCALE, -15.0, 15.0).astype(mybir.dt.np(DT_C))


def _np_a(a):
    return np.ascontiguousarray(a, dtype=mybir.dt.np(DT_A))


def _shard_inputs(x, cache_k, cache_v, W_q, W_k, W_v, W_o):
    """Build per-core input maps with the on-device layouts."""
    x = np.asarray(x, dtype=np.float32)
    cache_k = np.asarray(cache_k, dtype=np.float32)
    cache_v = np.asarray(cache_v, dtype=np.float32)
    W_q = np.asarray(W_q, dtype=np.float32)
    W_k = np.asarray(W_k, dtype=np.float32)
    # fold the KV_SCALE bookkeeping into the projection weights:
    #   vnew must carry the same x2 as the stored V cache -> W_v * 2
    #   the x2 on the whole context is cancelled at the end -> W_o * 0.5
    W_v = np.asarray(W_v, dtype=np.float32) * KV_SCALE
    W_o = np.asarray(W_o, dtype=np.float32) * (1.0 / KV_SCALE)

    # xt[p, kk*B + b] = x[b, 0, kk*128 + p]  (shared by all cores)
    xt = _np_a(
        x[:, 0, :].T.reshape(KT, 128, B).transpose(1, 0, 2).reshape(128, KT * B)
    )

    in_maps = []
    for c in range(N_CORES):
        rows = slice(c * HPC * D, (c + 1) * HPC * D)
        # K^T per (h,b): [d, s]; pack CH batches along free dim per chunk
        k_c = cache_k[:, c * HPC:(c + 1) * HPC]          # [B, HPC, S, D]
        k_t = k_c.transpose(1, 0, 3, 2)                  # [HPC, B, D, S]
        k_t = k_t.reshape(HPC, NG, CH, 128, S).transpose(0, 1, 3, 2, 4)
        k_t = k_t.reshape(HPC, NG, 128, CH * S)
        # V natural per (h,b): rows s in tiles of 128 on partitions:
        # v[h, b, p, si*128 + d] = V[si*128 + p, d]
        v_c = cache_v[:, c * HPC:(c + 1) * HPC]          # [B, HPC, S, D]
        v_t = v_c.transpose(1, 0, 2, 3)                  # [HPC, B, S, D]
        v_t = v_t.reshape(HPC, B, ST, 128, D).transpose(0, 1, 3, 2, 4)
        v_t = v_t.reshape(HPC, NG, CH, 128, ST * D).transpose(0, 1, 3, 2, 4)
        v_t = v_t.reshape(HPC, NG, 128, CH * S)

        def wslice(W):
            # w[p, kk*HPC*D + h*D + m] = W[rows][h*D + m, kk*128 + p]
            wr = W[rows, :]                              # [HPC*D, DM]
            wr = wr.reshape(HPC * D, KT, 128).transpose(2, 1, 0)   # [p, kk, m]
            return _np_a(wr.reshape(128, KT * HPC * D))

        # wo[p, h*DM + j] = W_o[j, c*HPC*D + h*128 + p]
        wo = W_o[:, rows].T.reshape(HPC, 128, DM).transpose(1, 0, 2)
        wo = _np_a(wo.reshape(128, HPC * DM))

        in_maps.append({
            "kt": _np_c(k_t),
            "vv": _np_c(v_t),
            "wq": wslice(W_q),
            "wk": wslice(W_k),
            "wv": wslice(W_v),
            "wo": wo,
            "xt": xt,
        })
    return in_maps


def run_sharded(inputs, trace=False):
    """Run the SPMD kernel; returns BassKernelResults."""
    nc = _get_kernel()
    in_maps = _shard_inputs(**inputs)
    res = run_bass_kernel_spmd(nc, in_maps, core_ids=list(range(N_CORES)),
                               trace=trace)
    return res


def kernel(x, cache_k, cache_v, W_q, W_k, W_v, W_o) -> np.ndarray:
    res = run_sharded(dict(x=x, cache_k=cache_k, cache_v=cache_v,
                           W_q=W_q, W_k=W_k, W_v=W_v, W_o=W_o))
    total = np.zeros((B, DM), dtype=np.float32)
    for c in range(N_CORES):
        total += res.results[c]["out"]
    return total.reshape(B, 1, DM)
